# revision 36
# baseline (speedup 1.0000x reference)
"""Transformer block (LN -> 12-head causal attention -> residual -> LN -> MLP
-> residual) for B=4, T=2048, C=768 on 8 trn2 NeuronCores.

Sharding: core = (batch, token-half). Each core handles one batch's K/V in
full and produces the final output for half the tokens (even or odd 128-token
blocks, which balances the causal-attention triangle). No collectives; all
per-core structural differences are carried in input *data* (host-gathered
xTm, causal-boundary mask tiles) so a single SPMD program runs on all 8
cores.

On-chip layout is feature-major ("transposed", [C, T]). LN statistics are
computed with ones-vector matmuls on the tensor engine and the LN1+QKV
pipeline is interleaved per 512-token group so the PE never starves.
Attention processes all 1024 owned query columns per head in one pass
(1024-wide exp tiles halve the scalar-engine instruction count); softmax
row-sums ride a ones column appended to V and are inverted with the fast
Newton-Raphson reciprocal.
"""

import math
import os
import sys

for _p in ("/opt/trn_rl_repo", "/root/.axon_site/_ro/trn_rl_repo"):
    if os.path.isdir(_p) and _p not in sys.path:
        sys.path.append(_p)

import numpy as np
import ml_dtypes

import concourse.bacc as bacc
import concourse.tile as tile
import concourse.mybir as mybir
from concourse import bass_utils
from concourse.alu_op_type import AluOpType
from concourse.tile_rust import add_dep_helper

BF = mybir.dt.bfloat16
FP = mybir.dt.float32
AF = mybir.ActivationFunctionType

B, T, C, H, HD = 4, 2048, 768, 12, 64
EPS = 1e-5
SHIFT = 40.0  # constant softmax shift: exp(s - SHIFT); exact softmax
NP = C // 128  # 6 feature partition-tiles
NT = T // 128  # 16 token blocks
TM = T // 2    # 1024 tokens owned per core
NG = 4         # 512-token groups
bf16 = ml_dtypes.bfloat16

_cache = {}


def _build(debug=False):
    nc = bacc.Bacc("TRN2", target_bir_lowering=False, debug=False)
    d_xT = nc.dram_tensor("xT", [C, T], FP, kind="ExternalInput").ap()
    d_xTm = nc.dram_tensor("xTm", [C, TM], FP, kind="ExternalInput").ap()
    d_wqkvp = nc.dram_tensor("wqkvp", [C, 4 * C], BF, kind="ExternalInput").ap()
    d_w1 = nc.dram_tensor("w1p", [C, 4 * C], BF, kind="ExternalInput").ap()
    d_w2 = nc.dram_tensor("w2p", [C, 4 * C], BF, kind="ExternalInput").ap()
    d_bias = nc.dram_tensor("biasp", [C, 9], FP, kind="ExternalInput").ap()
    d_bvrow = nc.dram_tensor("bvrow", [1, C], FP, kind="ExternalInput").ap()
    d_masks = nc.dram_tensor("masks", [256, 128], BF, kind="ExternalInput").ap()
    d_out = nc.dram_tensor("outT", [C, TM], FP, kind="ExternalOutput").ap()
    if debug is True:
        debug = ["h", "hm", "KT", "QT", "V", "attnT", "xmid"]
    debug = debug or []
    dbg = {}
    if "h" in debug:
        dbg["h"] = nc.dram_tensor("dbg_h", [C, T], BF, kind="ExternalOutput").ap()
    if "hm" in debug:
        dbg["hm"] = nc.dram_tensor("dbg_hm", [C, TM], BF, kind="ExternalOutput").ap()
    if "KT" in debug:
        dbg["KT"] = nc.dram_tensor("dbg_KT", [C, T], BF, kind="ExternalOutput").ap()
    if "QT" in debug:
        dbg["QT"] = nc.dram_tensor("dbg_QT", [C, TM], BF, kind="ExternalOutput").ap()
    if "V" in debug:
        dbg["V"] = nc.dram_tensor("dbg_V", [T, H * 65], BF, kind="ExternalOutput").ap()
    if "attnT" in debug:
        dbg["attnT"] = nc.dram_tensor("dbg_attnT", [C, TM], BF, kind="ExternalOutput").ap()
    if "xmid" in debug:
        dbg["xmid"] = nc.dram_tensor("dbg_xmid", [C, TM], FP, kind="ExternalOutput").ap()

    with tile.TileContext(nc) as tc:
        _body(nc, tc, d_xT, d_xTm, d_wqkvp, d_w1, d_w2, d_bias, d_bvrow,
              d_masks, d_out, dbg)
    nc.compile()
    return nc


def _ln_smalls(nc, small, stats, ncols, eps_c, a_dst, c_dst):
    """From accumulated [33, ncols] stats (row 0 = sum, row 32 = sumsq),
    produce bf16 [1, ncols] rows a5b (1/std) and c5b (-mu/std)."""
    mu = small.tile([1, ncols], FP, tag=f"mu{ncols}", name="mu")
    nc.scalar.mul(mu[:], stats[0:1, :], 1.0 / C)
    m2 = small.tile([1, ncols], FP, tag=f"m2{ncols}", name="m2")
    nc.scalar.mul(m2[:], stats[32:33, :], 1.0 / C)
    var = small.tile([1, ncols], FP, tag=f"va{ncols}", name="va")
    nc.vector.tensor_mul(var[:], mu[:], mu[:])
    nc.vector.tensor_sub(var[:], m2[:], var[:])
    std = small.tile([1, ncols], FP, tag=f"sd{ncols}", name="sd")
    nc.scalar.activation(std[:], var[:], AF.Sqrt, bias=eps_c[0:1, 0:1])
    a5 = small.tile([1, ncols], FP, tag=f"a5{ncols}", name="a5")
    nc.vector.reciprocal_approx_accurate(a5[:], std[:], var[:])
    c5 = small.tile([1, ncols], FP, tag=f"c5{ncols}", name="c5")
    nc.vector.scalar_tensor_tensor(c5[:], mu[:], -1.0, a5[:],
                                   AluOpType.mult, AluOpType.mult)
    nc.vector.tensor_copy(a_dst[:], a5[:])
    nc.vector.tensor_copy(c_dst[:], c5[:])


def _body(nc, tc, d_xT, d_xTm, d_wqkvp, d_w1, d_w2, d_bias, d_bvrow,
          d_masks, d_out, dbg={}):
    from contextlib import ExitStack

    es = ExitStack()
    g_const = es.enter_context(tc.tile_pool(name="const", bufs=1))
    g_xmid = es.enter_context(tc.tile_pool(name="xmid", bufs=1))
    # bf16 residual trunk: halves SBUF so MLP weights can coexist with K/Q/V
    xmid = [g_xmid.tile([128, TM], BF, tag=f"xm{i}", name=f"xm{i}") for i in range(NP)]
    kqv_stack = ExitStack()
    w_es = ExitStack()
    g_kqv = kqv_stack.enter_context(tc.tile_pool(name="kqv", bufs=1))
    g_w = w_es.enter_context(tc.tile_pool(name="wqkvp", bufs=1))

    # ---- constants ----
    ones_bf = g_const.tile([128, 1], BF, tag="ones_bf", name="ones_bf")
    nc.vector.memset(ones_bf[:], 1.0)
    eps_c = g_const.tile([128, 1], FP, tag="eps_c", name="eps_c")
    nc.vector.memset(eps_c[:], EPS)
    shift_c = g_const.tile([128, 1], FP, tag="shift_c", name="shift_c")
    nc.vector.memset(shift_c[:], -SHIFT)
    bias_sb = [g_const.tile([128, 9], FP, tag=f"bias{f}", name=f"bias{f}") for f in range(NP)]
    for f in range(NP):
        nc.sync.dma_start(bias_sb[f][:], d_bias[f * 128:(f + 1) * 128, :])
    mask_a = g_const.tile([128, 128], BF, tag="mask_a", name="mask_a")
    nc.sync.dma_start(mask_a[:], d_masks[0:128, :])
    mask_b = g_const.tile([128, 128], BF, tag="mask_b", name="mask_b")
    nc.sync.dma_start(mask_b[:], d_masks[128:256, :])
    bv_row = g_const.tile([1, C], FP, tag="bv_row", name="bv_row")
    nc.sync.dma_start(bv_row[:], d_bvrow[:])
    bv_rb = g_const.tile([1, C], BF, tag="bv_rb", name="bv_rb")
    nc.vector.tensor_copy(bv_rb[:], bv_row[:])
    bv_bc = g_const.tile([128, C], BF, tag="bv_bc", name="bv_bc")
    nc.gpsimd.partition_broadcast(bv_bc[:], bv_rb[:])

    # ---- weights for attention part ----
    w_sb = [g_w.tile([128, 4 * C], BF, tag=f"w{c}", name=f"w{c}") for c in range(NP)]
    for c in range(NP):
        nc.sync.dma_start(w_sb[c][:], d_wqkvp[c * 128:(c + 1) * 128, :])

    # ---- persistent activation storage ----
    KT = [g_kqv.tile([128, T], BF, tag=f"KT{i}", name=f"KT{i}") for i in range(NP)]
    QT = [g_kqv.tile([128, TM], BF, tag=f"QT{i}", name=f"QT{i}") for i in range(NP)]
    Vsb = [g_kqv.tile([128, H * 65], BF, tag=f"V{t}", name=f"V{t}") for t in range(NT)]

    # ================= LN1 + QKV, pipelined per 512-token group =============
    ln_es = ExitStack()
    g_roll = ln_es.enter_context(tc.tile_pool(name="lnroll", bufs=2))
    g_bc = ln_es.enter_context(tc.tile_pool(name="lnbc", bufs=2))
    g_small = ln_es.enter_context(tc.tile_pool(name="lnsmall", bufs=1))
    sps = ln_es.enter_context(tc.tile_pool(name="statps", bufs=2, space="PSUM"))
    gps = ln_es.enter_context(tc.tile_pool(name="gemmps", bufs=3, space="PSUM"))

    def emit_ln_group(g):
        """LN1 stats + h/hm for token group g (cols g*512..(g+1)*512 of T,
        owned cols g*256..(g+1)*256 of TM). Returns per-group h/hm tiles."""
        csl = slice(g * 512, (g + 1) * 512)
        msl = slice(g * 256, (g + 1) * 256)
        stats = sps.tile([33, 512], FP, tag="stf", name="stf")
        mstats = sps.tile([33, 256], FP, tag="stm", name="stm")
        xbs, xbms = [], []
        for ci in range(NP):
            xt = g_roll.tile([128, 512], FP, tag="xr", name="xr")
            nc.sync.dma_start(xt[:], d_xT[ci * 128:(ci + 1) * 128, csl])
            xb = g_roll.tile([128, 512], BF, tag="xb", name="xb", bufs=8)
            nc.vector.tensor_copy(xb[:], xt[:])
            sq = g_roll.tile([128, 512], BF, tag="sq", name="sq")
            nc.vector.tensor_mul(sq[:], xb[:], xb[:])
            nc.tensor.matmul(stats[0:1, :], ones_bf[:], xb[:],
                             start=(ci == 0), stop=(ci == NP - 1),
                             skip_group_check=True)
            nc.tensor.matmul(stats[32:33, :], ones_bf[:], sq[:],
                             start=(ci == 0), stop=(ci == NP - 1),
                             skip_group_check=True)
            xbs.append(xb)
            xtm = g_roll.tile([128, 256], FP, tag="xrm", name="xrm")
            nc.sync.dma_start(xtm[:], d_xTm[ci * 128:(ci + 1) * 128, msl])
            xbm = g_roll.tile([128, 256], BF, tag="xbm", name="xbm", bufs=8)
            nc.vector.tensor_copy(xbm[:], xtm[:])
            sqm = g_roll.tile([128, 256], BF, tag="sqm", name="sqm")
            nc.vector.tensor_mul(sqm[:], xbm[:], xbm[:])
            nc.tensor.matmul(mstats[0:1, :], ones_bf[:], xbm[:],
                             start=(ci == 0), stop=(ci == NP - 1),
                             skip_group_check=True)
            nc.tensor.matmul(mstats[32:33, :], ones_bf[:], sqm[:],
                             start=(ci == 0), stop=(ci == NP - 1),
                             skip_group_check=True)
            xbms.append(xbm)
        a_row = g_small.tile([1, 512], BF, tag="a_row", name="a_row")
        c_row = g_small.tile([1, 512], BF, tag="c_row", name="c_row")
        _ln_smalls(nc, g_small, stats, 512, eps_c, a_row, c_row)
        a_bc = g_bc.tile([128, 512], BF, tag="a_bc", name="a_bc")
        c_bc = g_bc.tile([128, 512], BF, tag="c_bc", name="c_bc")
        nc.gpsimd.partition_broadcast(a_bc[:], a_row[:])
        nc.gpsimd.partition_broadcast(c_bc[:], c_row[:])
        am_row = g_small.tile([1, 256], BF, tag="am_row", name="am_row")
        cm_row = g_small.tile([1, 256], BF, tag="cm_row", name="cm_row")
        _ln_smalls(nc, g_small, mstats, 256, eps_c, am_row, cm_row)
        am_bc = g_bc.tile([128, 256], BF, tag="am_bc", name="am_bc")
        cm_bc = g_bc.tile([128, 256], BF, tag="cm_bc", name="cm_bc")
        nc.gpsimd.partition_broadcast(am_bc[:], am_row[:])
        nc.gpsimd.partition_broadcast(cm_bc[:], cm_row[:])
        hg, hmg = [], []
        for ci in range(NP):
            tmp = g_roll.tile([128, 512], BF, tag="h_tmp", name="h_tmp")
            nc.vector.tensor_mul(tmp[:], xbs[ci][:], a_bc[:])
            hgc = g_roll.tile([128, 512], BF, tag=f"h{ci}", name="hgc")
            nc.vector.tensor_add(hgc[:], tmp[:], c_bc[:])
            hg.append(hgc)
            tmpm = g_roll.tile([128, 256], BF, tag="hm_tmp", name="hm_tmp")
            nc.vector.tensor_mul(tmpm[:], xbms[ci][:], am_bc[:])
            hmgc = g_roll.tile([128, 256], BF, tag=f"hm{ci}", name="hmgc")
            nc.vector.tensor_add(hmgc[:], tmpm[:], cm_bc[:])
            hmg.append(hmgc)
        if "h" in dbg:
            for ci in range(NP):
                nc.sync.dma_start(dbg["h"][ci * 128:(ci + 1) * 128, csl], hg[ci][:])
        if "hm" in dbg:
            for ci in range(NP):
                nc.sync.dma_start(dbg["hm"][ci * 128:(ci + 1) * 128, msl], hmg[ci][:])
        return hg, hmg

    def emit_qkv_group(g, hg, hmg):
        """K/Q/V GEMMs for token group g."""
        csl = slice(g * 512, (g + 1) * 512)
        msl = slice(g * 256, (g + 1) * 256)
        # K^T [C, T]: lhsT = wk tile, rhs = h
        for f in range(NP):
            ps = gps.tile([128, 512], FP, tag="ps", name="ps")
            for c in range(NP):
                nc.tensor.matmul(ps[:], w_sb[c][:, C + f * 128:C + (f + 1) * 128],
                                 hg[c][:], start=(c == 0), stop=(c == NP - 1))
            nc.scalar.activation(KT[f][:, csl], ps[:], AF.Identity,
                                 bias=bias_sb[f][:, 1:2])
        # Q^T [C, TM] from h_mine
        for f in range(NP):
            ps = gps.tile([128, 512], FP, tag="ps", name="ps")
            for c in range(NP):
                nc.tensor.matmul(ps[:, 0:256], w_sb[c][:, f * 128:(f + 1) * 128],
                                 hmg[c][:], start=(c == 0), stop=(c == NP - 1))
            nc.scalar.activation(QT[f][:, msl], ps[:, 0:256], AF.Identity,
                                 bias=bias_sb[f][:, 0:1])
        # V natural [T, C] (+ ones col per head): lhsT = h tile, rhs = wv
        for t in range(4):
            v3 = Vsb[4 * g + t][:].rearrange("p (h d) -> p h d", d=65)
            nc.vector.memset(v3[:, :, 64:65], 1.0)
            for fs in range(2):
                n = 512 if fs == 0 else 256
                nh = n // 64
                ps = gps.tile([128, 512], FP, tag="ps", name="ps")
                for c in range(NP):
                    nc.tensor.matmul(ps[:, 0:n], hg[c][:, t * 128:(t + 1) * 128],
                                     w_sb[c][:, 2 * C + fs * 512:2 * C + fs * 512 + n],
                                     start=(c == 0), stop=(c == NP - 1))
                nc.vector.scalar_tensor_tensor(
                    v3[:, fs * 8:fs * 8 + nh, 0:64],
                    ps[:, 0:n].rearrange("p (h d) -> p h d", d=64),
                    0.0,
                    bv_bc[:, fs * 512:fs * 512 + n].rearrange("p (h d) -> p h d", d=64),
                    AluOpType.add, AluOpType.add)

    cur = emit_ln_group(0)
    for g in range(NG):
        nxt = emit_ln_group(g + 1) if g + 1 < NG else None
        emit_qkv_group(g, *cur)
        cur = nxt

    for c in range(NP):
        if "KT" in dbg:
            nc.sync.dma_start(dbg["KT"][c * 128:(c + 1) * 128, :], KT[c][:])
        if "QT" in dbg:
            nc.sync.dma_start(dbg["QT"][c * 128:(c + 1) * 128, :], QT[c][:])
    if "V" in dbg:
        for t in range(NT):
            nc.sync.dma_start(dbg["V"][t * 128:(t + 1) * 128, :], Vsb[t][:])
    ln_es.close()

    # proj weights copied out so the big QKV weight block can be freed
    wproj = [g_kqv.tile([128, C], BF, tag=f"wp{c}", name=f"wp{c}") for c in range(NP)]
    for c in range(NP):
        nc.vector.tensor_copy(wproj[c][:], w_sb[c][:, 3 * C:4 * C])
    w_es.close()

    attnT_es = ExitStack()
    g_attnT = attnT_es.enter_context(tc.tile_pool(name="attnT", bufs=1))
    attnT = [g_attnT.tile([128, TM], BF, tag=f"aT{i}", name=f"aT{i}") for i in range(NP)]

    # ========== fused attention + proj + LN2 + MLP ==========
    fus_es = ExitStack()
    g_w1 = fus_es.enter_context(tc.tile_pool(name="w1p", bufs=1))
    g_h2 = fus_es.enter_context(tc.tile_pool(name="h2p", bufs=1))
    g_r = fus_es.enter_context(tc.tile_pool(name="rp", bufs=1))
    g_roll2 = fus_es.enter_context(tc.tile_pool(name="mlproll", bufs=2))
    g_bc2 = fus_es.enter_context(tc.tile_pool(name="mlpbc", bufs=2))
    g_small2 = fus_es.enter_context(tc.tile_pool(name="mlpsmall", bufs=1))
    g_wei = fus_es.enter_context(tc.tile_pool(name="wei", bufs=2))
    g_asc = fus_es.enter_context(tc.tile_pool(name="ascratch", bufs=1))
    g_rb = fus_es.enter_context(tc.tile_pool(name="rbpool", bufs=1))
    ps_s_pool = fus_es.enter_context(tc.tile_pool(name="sps2", bufs=2, space="PSUM"))
    ps_a_pool = fus_es.enter_context(tc.tile_pool(name="aps", bufs=2, space="PSUM"))
    gps3 = fus_es.enter_context(tc.tile_pool(name="mps", bufs=3, space="PSUM"))
    sps3 = fus_es.enter_context(tc.tile_pool(name="statps3", bufs=1, space="PSUM"))

    w1_sb = [g_w1.tile([128, 4 * C], BF, tag=f"w1_{c}", name=f"w1_{c}") for c in range(NP)]
    for c in range(NP):
        nc.sync.dma_start(w1_sb[c][:], d_w1[c * 128:(c + 1) * 128, :])

    def att_head(hh, g):
        ht, hp = hh // 2, (hh % 2) * 64
        smax = 8 + 8 * g
        pa = ps_a_pool.tile([65, 512], FP, tag="pa", name="pa")
        for sb in range(smax):
            jmin = max(0, math.ceil((sb - 1 - 8 * g) / 2))
            c0 = jmin * 128
            ps = ps_s_pool.tile([128, 512], FP, tag="ps", name="ps")
            nc.tensor.matmul(ps[:, c0:512],
                             KT[ht][hp:hp + 64, sb * 128:(sb + 1) * 128],
                             QT[ht][hp:hp + 64, g * 512 + c0:(g + 1) * 512],
                             start=True, stop=True)
            wei = g_wei.tile([128, 512], BF, tag="wei", name="wei")
            nc.scalar.activation(wei[:, c0:512], ps[:, c0:512], AF.Exp,
                                 bias=shift_c[:])
            if (sb - 8 * g) % 2 == 0:
                ja = (sb - 8 * g) // 2
                if 0 <= ja < 4:
                    nc.vector.tensor_mul(wei[:, ja * 128:(ja + 1) * 128],
                                         wei[:, ja * 128:(ja + 1) * 128],
                                         mask_a[:])
            else:
                jb = (sb - 1 - 8 * g) // 2
                if 0 <= jb < 4:
                    nc.vector.tensor_mul(wei[:, jb * 128:(jb + 1) * 128],
                                         wei[:, jb * 128:(jb + 1) * 128],
                                         mask_b[:])
            nc.tensor.matmul(pa[:, c0:512], Vsb[sb][:, hh * 65:(hh + 1) * 65],
                             wei[:, c0:512], start=(sb == 0),
                             stop=(sb == smax - 1), skip_group_check=True)
        sumrow = g_asc.tile([1, 512], FP, tag="sumrow", name="sumrow")
        nc.vector.tensor_copy(sumrow[:], pa[64:65, :])
        recip = g_asc.tile([1, 512], FP, tag="recip", name="recip")
        scr = g_asc.tile([1, 512], FP, tag="scr", name="scr")
        nc.vector.reciprocal_approx_accurate(recip[:], sumrow[:], scr[:])
        rb = g_rb.tile([64, 512], FP, tag="rb", name="rb")
        nc.gpsimd.partition_broadcast(rb[:], recip[:])
        nc.vector.tensor_mul(attnT[ht][hp:hp + 64, g * 512:(g + 1) * 512],
                             pa[0:64, :], rb[:])

    def emit_proj(g):
        gsl = slice(g * 512, (g + 1) * 512)
        for f in range(NP):
            ps = gps3.tile([128, 512], FP, tag="ps", name="ps")
            for c in range(NP):
                nc.tensor.matmul(ps[:], wproj[c][:, f * 128:(f + 1) * 128],
                                 attnT[c][:, gsl],
                                 start=(c == 0), stop=(c == NP - 1))
            xr = g_roll2.tile([128, 512], FP, tag="xr", name="xr")
            nc.sync.dma_start(xr[:], d_xTm[f * 128:(f + 1) * 128, gsl])
            nc.vector.scalar_tensor_tensor(
                xmid[f][:, gsl], ps[:], bias_sb[f][:, 3:4],
                xr[:], AluOpType.add, AluOpType.add)

    h2 = {}

    def emit_ln2(g):
        gsl = slice(g * 512, (g + 1) * 512)
        stats = sps3.tile([33, 512], FP, tag="st2", name="st2")
        for ci in range(NP):
            sq = g_roll2.tile([128, 512], BF, tag="sq2", name="sq2")
            nc.vector.tensor_mul(sq[:], xmid[ci][:, gsl], xmid[ci][:, gsl])
            nc.tensor.matmul(stats[0:1, :], ones_bf[:], xmid[ci][:, gsl],
                             start=(ci == 0), stop=(ci == NP - 1),
                             skip_group_check=True)
            nc.tensor.matmul(stats[32:33, :], ones_bf[:], sq[:],
                             start=(ci == 0), stop=(ci == NP - 1),
                             skip_group_check=True)
        a_row2 = g_small2.tile([1, 512], BF, tag="a_row2", name="a_row2")
        c_row2 = g_small2.tile([1, 512], BF, tag="c_row2", name="c_row2")
        _ln_smalls(nc, g_small2, stats, 512, eps_c, a_row2, c_row2)
        a_bc2 = g_bc2.tile([128, 512], BF, tag="a2bc", name="a2bc")
        c_bc2 = g_bc2.tile([128, 512], BF, tag="c2bc", name="c2bc")
        nc.gpsimd.partition_broadcast(a_bc2[:], a_row2[:])
        nc.gpsimd.partition_broadcast(c_bc2[:], c_row2[:])
        hts = []
        for ci in range(NP):
            tmp = g_roll2.tile([128, 512], BF, tag="h2tmp", name="h2tmp")
            nc.vector.tensor_mul(tmp[:], xmid[ci][:, gsl], a_bc2[:])
            hh2 = g_h2.tile([128, 512], BF, tag=f"h2_{ci}", name=f"h2_{ci}")
            nc.vector.tensor_add(hh2[:], tmp[:], c_bc2[:])
            hts.append(hh2)
        h2[g] = hts

    r_tiles = {}

    def emit_fc1(g, m0, m1):
        for m in range(m0, m1):
            ps = gps3.tile([128, 512], FP, tag="ps", name="ps")
            for c in range(NP):
                nc.tensor.matmul(ps[:], w1_sb[c][:, m * 128:(m + 1) * 128],
                                 h2[g][c][:],
                                 start=(c == 0), stop=(c == NP - 1))
            r = g_r.tile([128, 512], BF, tag=f"r{m}", name=f"r{m}")
            nc.vector.tensor_scalar(r[:], ps[:],
                                    bias_sb[m % 6][:, 5 + m // 6:6 + m // 6],
                                    0.0, AluOpType.add, AluOpType.max)
            r_tiles[g, m] = r

    def emit_fc2(g):
        gsl = slice(g * 512, (g + 1) * 512)
        for f in range(NP):
            ps = gps3.tile([128, 512], FP, tag="ps", name="ps")
            for m in range(24):
                nc.tensor.matmul(ps[:], w2_slice(m, f), r_tiles[g, m][:],
                                 start=(m == 0), stop=(m == 23))
            ot = g_roll2.tile([128, 512], FP, tag="ot", name="ot")
            nc.vector.scalar_tensor_tensor(ot[:], ps[:], bias_sb[f][:, 4:5],
                                           xmid[f][:, gsl],
                                           AluOpType.add, AluOpType.add)
            nc.sync.dma_start(d_out[f * 128:(f + 1) * 128, gsl], ot[:])

    # attention group 0, then group 1 interleaved with proj/LN2/fc1 of group 0
    # so the PE keeps dense work (and full clock) while the exp chain runs
    for hh in range(H):
        att_head(hh, 0)
    work = [lambda: emit_proj(0), lambda: emit_ln2(0),
            lambda: emit_fc1(0, 0, 6), lambda: emit_fc1(0, 6, 12),
            lambda: emit_fc1(0, 12, 18), lambda: emit_fc1(0, 18, 24)]
    wi = 0
    for hh in range(H):
        att_head(hh, 1)
        if hh >= 2 and hh % 2 == 0 and wi < len(work):
            work[wi]()
            wi += 1
    while wi < len(work):
        work[wi]()
        wi += 1
    if "attnT" in dbg:
        for c in range(NP):
            nc.sync.dma_start(dbg["attnT"][c * 128:(c + 1) * 128, :], attnT[c][:])

    # w2 loads into the KT/QT buffers (pool-tag reuse) once attention is done
    w2k = [g_kqv.tile([128, 2048], BF, tag=f"KT{i}", name=f"w2k{i}") for i in range(6)]
    w2q = [g_kqv.tile([128, 1024], BF, tag=f"QT{i}", name=f"w2q{i}") for i in range(6)]
    for k in range(6):
        g0, g1 = k * 2048, (k + 1) * 2048
        st = g0
        while st < g1:
            c = st // 3072
            en = min(g1, (c + 1) * 3072)
            nc.sync.dma_start(w2k[k][:, st - g0:en - g0],
                              d_w2[c * 128:(c + 1) * 128,
                                   st - c * 3072:en - c * 3072])
            st = en
    for i in range(6):
        g0 = 12288 + 1024 * i
        c = g0 // 3072
        nc.sync.dma_start(w2q[i][:], d_w2[c * 128:(c + 1) * 128,
                                          g0 - c * 3072:g0 - c * 3072 + 1024])

    def w2_slice(m, f):
        g0 = (m // 4) * 3072 + (m % 4) * 768 + f * 128
        if g0 < 12288:
            return w2k[g0 // 2048][:, g0 % 2048:g0 % 2048 + 128]
        rr = g0 - 12288
        return w2q[rr // 1024][:, rr % 1024:rr % 1024 + 128]

    emit_fc2(0)
    emit_proj(1)
    emit_ln2(1)
    emit_fc1(1, 0, 24)
    emit_fc2(1)
    if "xmid" in dbg:
        for c in range(NP):
            nc.sync.dma_start(dbg["xmid"][c * 128:(c + 1) * 128, :], xmid[c][:])
    fus_es.close()
    attnT_es.close()
    kqv_stack.close()
    es.close()


# ---------------------------------------------------------------------------
# host side
# ---------------------------------------------------------------------------

def _mycols(half):
    blocks = np.arange(8) * 2 + half
    return (blocks[:, None] * 128 + np.arange(128)[None, :]).reshape(-1)


def _prep_inputs(x, wq, bq, wk, bk, wv, bv, w_proj, b_proj, w1, b1, w2, b2,
                 g1, beta1, g2, beta2):
    x = np.asarray(x, np.float32)
    wq_f = np.ascontiguousarray(np.transpose(np.asarray(wq, np.float32), (1, 0, 2)).reshape(C, C))
    wk_f = np.ascontiguousarray(np.transpose(np.asarray(wk, np.float32), (1, 0, 2)).reshape(C, C))
    wv_f = np.ascontiguousarray(np.transpose(np.asarray(wv, np.float32), (1, 0, 2)).reshape(C, C))
    g1 = np.asarray(g1, np.float32); beta1 = np.asarray(beta1, np.float32)
    g2 = np.asarray(g2, np.float32); beta2 = np.asarray(beta2, np.float32)
    w1 = np.asarray(w1, np.float32); w2 = np.asarray(w2, np.float32)
    w_proj = np.asarray(w_proj, np.float32)

    wq_g = g1[:, None] * wq_f
    wk_g = g1[:, None] * wk_f
    wv_g = g1[:, None] * wv_f
    bq_f = beta1 @ wq_f + np.asarray(bq, np.float32).reshape(-1)
    bk_f = beta1 @ wk_f + np.asarray(bk, np.float32).reshape(-1)
    bv_f = beta1 @ wv_f + np.asarray(bv, np.float32).reshape(-1)
    w1_g = g2[:, None] * w1
    b1_f = beta2 @ w1 + np.asarray(b1, np.float32)

    wqkvp = np.concatenate([wq_g, wk_g, wv_g, w_proj], axis=1).astype(bf16)
    w1p = w1_g.astype(bf16)
    w2p = np.ascontiguousarray(
        w2.reshape(6, 4, 128, C).transpose(0, 2, 1, 3).reshape(C, 4 * C)).astype(bf16)

    biasp = np.zeros((C, 9), np.float32)
    biasp[:, 0] = bq_f
    biasp[:, 1] = bk_f
    biasp[:, 2] = bv_f
    biasp[:, 3] = np.asarray(b_proj, np.float32)
    biasp[:, 4] = np.asarray(b2, np.float32)
    biasp[:, 5:9] = b1_f.reshape(4, C).T
    bvrow = bv_f.reshape(1, C).astype(np.float32)

    tri = np.tril(np.ones((128, 128), np.float32)).T  # [s, q]: 1 iff s <= q
    in_maps = []
    for core in range(8):
        b, half = core // 2, core % 2
        xT = np.ascontiguousarray(x[b].T)
        xTm = np.ascontiguousarray(xT[:, _mycols(half)])
        masks = np.zeros((256, 128), np.float32)
        if half == 0:
            masks[0:128] = tri
            masks[128:256] = 0.0
        else:
            masks[0:128] = 1.0
            masks[128:256] = tri
        in_maps.append({
            "xT": xT, "xTm": xTm,
            "wqkvp": wqkvp, "w1p": w1p, "w2p": w2p,
            "biasp": biasp, "bvrow": bvrow,
            "masks": masks.astype(bf16),
        })
    return in_maps


def _assemble(results, dtype):
    out = np.empty((B, T, C), dtype)
    for core in range(8):
        b, half = core // 2, core % 2
        out[b, _mycols(half), :] = results[core]["outT"].T
    return out


def kernel(**inputs):
    in_maps = _prep_inputs(**inputs)
    if "nc" not in _cache:
        _cache["nc"] = _build()
    res = bass_utils.run_bass_kernel_spmd(_cache["nc"], in_maps,
                                          core_ids=list(range(8)))
    return _assemble(res.results, np.asarray(inputs["x"]).dtype)


# revision 42
# speedup vs baseline: 1.0260x; 1.0260x over previous
"""Transformer block (LN -> 12-head causal attention -> residual -> LN -> MLP
-> residual) for B=4, T=2048, C=768 on 8 trn2 NeuronCores.

Sharding: core = (batch, token-half). Each core handles one batch's K/V in
full and produces the final output for half the tokens (even or odd 128-token
blocks, which balances the causal-attention triangle). No collectives; all
per-core structural differences are carried in input *data* (host-gathered
xTm, causal-boundary mask tiles) so a single SPMD program runs on all 8
cores.

On-chip layout is feature-major ("transposed", [C, T]). LN statistics are
computed with ones-vector matmuls on the tensor engine and the LN1+QKV
pipeline is interleaved per 512-token group so the PE never starves.
Attention processes all 1024 owned query columns per head in one pass
(1024-wide exp tiles halve the scalar-engine instruction count); softmax
row-sums ride a ones column appended to V and are inverted with the fast
Newton-Raphson reciprocal.
"""

import math
import os
import sys

for _p in ("/opt/trn_rl_repo", "/root/.axon_site/_ro/trn_rl_repo"):
    if os.path.isdir(_p) and _p not in sys.path:
        sys.path.append(_p)

import numpy as np
import ml_dtypes

import concourse.bacc as bacc
import concourse.tile as tile
import concourse.mybir as mybir
from concourse import bass_utils
from concourse.alu_op_type import AluOpType
from concourse.tile_rust import add_dep_helper

BF = mybir.dt.bfloat16
FP = mybir.dt.float32
AF = mybir.ActivationFunctionType

B, T, C, H, HD = 4, 2048, 768, 12, 64
EPS = 1e-5
SHIFT = 40.0  # constant softmax shift: exp(s - SHIFT); exact softmax
NP = C // 128  # 6 feature partition-tiles
NT = T // 128  # 16 token blocks
TM = T // 2    # 1024 tokens owned per core
NG = 4         # 512-token groups
bf16 = ml_dtypes.bfloat16

_cache = {}


def _build(debug=False):
    nc = bacc.Bacc("TRN2", target_bir_lowering=False, debug=False)
    d_xT = nc.dram_tensor("xT", [C, T], FP, kind="ExternalInput").ap()
    d_xTm = nc.dram_tensor("xTm", [C, TM], FP, kind="ExternalInput").ap()
    d_wqkvp = nc.dram_tensor("wqkvp", [C, 4 * C], BF, kind="ExternalInput").ap()
    d_w1 = nc.dram_tensor("w1p", [C, 4 * C], BF, kind="ExternalInput").ap()
    d_w2 = nc.dram_tensor("w2p", [C, 4 * C], BF, kind="ExternalInput").ap()
    d_bias = nc.dram_tensor("biasp", [C, 9], FP, kind="ExternalInput").ap()
    d_bvrow = nc.dram_tensor("bvrow", [1, C], FP, kind="ExternalInput").ap()
    d_masks = nc.dram_tensor("masks", [256, 128], BF, kind="ExternalInput").ap()
    d_out = nc.dram_tensor("outT", [C, TM], FP, kind="ExternalOutput").ap()
    if debug is True:
        debug = ["h", "hm", "KT", "QT", "V", "attnT", "xmid"]
    debug = debug or []
    dbg = {}
    if "h" in debug:
        dbg["h"] = nc.dram_tensor("dbg_h", [C, T], BF, kind="ExternalOutput").ap()
    if "hm" in debug:
        dbg["hm"] = nc.dram_tensor("dbg_hm", [C, TM], BF, kind="ExternalOutput").ap()
    if "KT" in debug:
        dbg["KT"] = nc.dram_tensor("dbg_KT", [C, T], BF, kind="ExternalOutput").ap()
    if "QT" in debug:
        dbg["QT"] = nc.dram_tensor("dbg_QT", [C, TM], BF, kind="ExternalOutput").ap()
    if "V" in debug:
        dbg["V"] = nc.dram_tensor("dbg_V", [T, H * 65], BF, kind="ExternalOutput").ap()
    if "attnT" in debug:
        dbg["attnT"] = nc.dram_tensor("dbg_attnT", [C, TM], BF, kind="ExternalOutput").ap()
    if "xmid" in debug:
        dbg["xmid"] = nc.dram_tensor("dbg_xmid", [C, TM], FP, kind="ExternalOutput").ap()

    with tile.TileContext(nc) as tc:
        _body(nc, tc, d_xT, d_xTm, d_wqkvp, d_w1, d_w2, d_bias, d_bvrow,
              d_masks, d_out, dbg)
    nc.compile()
    return nc


def _ln_smalls(nc, small, stats, ncols, eps_c, a_dst, c_dst):
    """From accumulated [33, ncols] stats (row 0 = sum, row 32 = sumsq),
    produce bf16 [1, ncols] rows a5b (1/std) and c5b (-mu/std)."""
    n = ncols
    mu = small.tile([1, 512], FP, tag="mu", name="mu")[0:1, 0:n]
    nc.scalar.mul(mu, stats[0:1, :], 1.0 / C)
    m2 = small.tile([1, 512], FP, tag="m2", name="m2")[0:1, 0:n]
    nc.scalar.mul(m2, stats[32:33, :], 1.0 / C)
    var = small.tile([1, 512], FP, tag="va", name="va")[0:1, 0:n]
    nc.vector.tensor_mul(var, mu, mu)
    nc.vector.tensor_sub(var, m2, var)
    std = small.tile([1, 512], FP, tag="sd", name="sd")[0:1, 0:n]
    nc.scalar.activation(std, var, AF.Sqrt, bias=eps_c[0:1, 0:1])
    a5 = small.tile([1, 512], FP, tag="a5", name="a5")[0:1, 0:n]
    nc.vector.reciprocal_approx_accurate(a5, std, var)
    c5 = small.tile([1, 512], FP, tag="c5", name="c5")[0:1, 0:n]
    nc.vector.scalar_tensor_tensor(c5, mu, -1.0, a5,
                                   AluOpType.mult, AluOpType.mult)
    nc.vector.tensor_copy(a_dst[:], a5)
    nc.vector.tensor_copy(c_dst[:], c5)


def _body(nc, tc, d_xT, d_xTm, d_wqkvp, d_w1, d_w2, d_bias, d_bvrow,
          d_masks, d_out, dbg={}):
    from contextlib import ExitStack

    es = ExitStack()
    g_const = es.enter_context(tc.tile_pool(name="const", bufs=1))
    g_xmid = es.enter_context(tc.tile_pool(name="xmid", bufs=1))
    # bf16 residual trunk: halves SBUF so MLP weights can coexist with K/Q/V
    xmid = [g_xmid.tile([128, TM], BF, tag=f"xm{i}", name=f"xm{i}") for i in range(NP)]
    kqv_stack = ExitStack()
    w_es = ExitStack()
    g_kqv = kqv_stack.enter_context(tc.tile_pool(name="kqv", bufs=1))

    # ---- constants ----
    ones_bf = g_const.tile([128, 1], BF, tag="ones_bf", name="ones_bf")
    nc.vector.memset(ones_bf[:], 1.0)
    eps_c = g_const.tile([128, 1], FP, tag="eps_c", name="eps_c")
    nc.vector.memset(eps_c[:], EPS)
    shift_c = g_const.tile([128, 1], FP, tag="shift_c", name="shift_c")
    nc.vector.memset(shift_c[:], -SHIFT)
    bias_sb = [g_const.tile([128, 9], FP, tag=f"bias{f}", name=f"bias{f}") for f in range(NP)]
    for f in range(NP):
        nc.sync.dma_start(bias_sb[f][:], d_bias[f * 128:(f + 1) * 128, :])
    mask_ab = g_const.tile([128, 256], BF, tag="mask_ab", name="mask_ab")
    nc.sync.dma_start(mask_ab[:, 0:128], d_masks[0:128, :])
    nc.sync.dma_start(mask_ab[:, 128:256], d_masks[128:256, :])
    bv_row = g_const.tile([1, C], FP, tag="bv_row", name="bv_row")
    nc.sync.dma_start(bv_row[:], d_bvrow[:])
    bv_rb = g_const.tile([1, C], BF, tag="bv_rb", name="bv_rb")
    nc.vector.tensor_copy(bv_rb[:], bv_row[:])
    bv_bc = g_const.tile([128, C], BF, tag="bv_bc", name="bv_bc")
    nc.gpsimd.partition_broadcast(bv_bc[:], bv_rb[:])

    # ---- persistent activation storage ----
    KT = [g_kqv.tile([128, T], BF, tag=f"KT{i}", name=f"KT{i}") for i in range(NP)]
    QT = [g_kqv.tile([128, TM], BF, tag=f"QT{i}", name=f"QT{i}") for i in range(NP)]
    Vsb = [g_kqv.tile([128, H * 65], BF, tag=f"V{t}", name=f"V{t}") for t in range(NT)]
    wproj = [g_kqv.tile([128, C], BF, tag=f"wp{c}", name=f"wp{c}") for c in range(NP)]

    # ---- pools created up-front in global LIFO order ----
    attnT_es = ExitStack()
    g_attnT = attnT_es.enter_context(tc.tile_pool(name="attnT", bufs=1))
    attnT = [g_attnT.tile([128, TM], BF, tag=f"aT{i}", name=f"aT{i}") for i in range(NP)]

    fusA_es = ExitStack()
    g_wei = fusA_es.enter_context(tc.tile_pool(name="wei", bufs=2))
    g_asc = fusA_es.enter_context(tc.tile_pool(name="ascratch", bufs=1))
    g_rb = fusA_es.enter_context(tc.tile_pool(name="rbpool", bufs=1))
    pa_pool = fusA_es.enter_context(tc.tile_pool(name="aps", bufs=1, space="PSUM"))
    ps0_es = ExitStack()
    ps0_pool = ps0_es.enter_context(tc.tile_pool(name="sps0", bufs=1, space="PSUM"))

    # ---- weights for attention part (pool above att pools so it can close
    # right after the QKV phase) ----
    g_w = w_es.enter_context(tc.tile_pool(name="wqkvp", bufs=1))
    w_sb = [g_w.tile([128, 4 * C], BF, tag=f"w{c}", name=f"w{c}") for c in range(NP)]
    for c in range(NP):
        nc.sync.dma_start(w_sb[c][:], d_wqkvp[c * 128:(c + 1) * 128, :])

    # ================= LN1 + QKV, pipelined per 512-token group =============
    ln_es = ExitStack()
    g_roll = ln_es.enter_context(tc.tile_pool(name="lnroll", bufs=2))
    g_bc = ln_es.enter_context(tc.tile_pool(name="lnbc", bufs=2))
    g_small = ln_es.enter_context(tc.tile_pool(name="lnsmall", bufs=1))
    sps = ln_es.enter_context(tc.tile_pool(name="statps", bufs=1, space="PSUM"))
    gps = ln_es.enter_context(tc.tile_pool(name="gemmps", bufs=3, space="PSUM"))

    def emit_ln_group(g):
        """LN1 stats + h/hm for token group g (cols g*512..(g+1)*512 of T,
        owned cols g*256..(g+1)*256 of TM). Returns per-group h/hm tiles."""
        csl = slice(g * 512, (g + 1) * 512)
        msl = slice(g * 256, (g + 1) * 256)
        stats = sps.tile([33, 512], FP, tag="stf", name="stf")
        mstats = sps.tile([33, 256], FP, tag="stm", name="stm")
        xbs, xbms = [], []
        for ci in range(NP):
            xt = g_roll.tile([128, 512], FP, tag="xr", name="xr")
            nc.sync.dma_start(xt[:], d_xT[ci * 128:(ci + 1) * 128, csl])
            xb = g_roll.tile([128, 512], BF, tag="xb", name="xb", bufs=6)
            nc.vector.tensor_copy(xb[:], xt[:])
            sq = g_roll.tile([128, 512], BF, tag="sq", name="sq")
            nc.vector.tensor_mul(sq[:], xb[:], xb[:])
            nc.tensor.matmul(stats[0:1, :], ones_bf[:], xb[:],
                             start=(ci == 0), stop=(ci == NP - 1),
                             skip_group_check=True)
            nc.tensor.matmul(stats[32:33, :], ones_bf[:], sq[:],
                             start=(ci == 0), stop=(ci == NP - 1),
                             skip_group_check=True)
            xbs.append(xb)
            xtm = g_roll.tile([128, 256], FP, tag="xrm", name="xrm")
            nc.sync.dma_start(xtm[:], d_xTm[ci * 128:(ci + 1) * 128, msl])
            xbm = g_roll.tile([128, 256], BF, tag="xbm", name="xbm", bufs=6)
            nc.vector.tensor_copy(xbm[:], xtm[:])
            sqm = g_roll.tile([128, 256], BF, tag="sqm", name="sqm")
            nc.vector.tensor_mul(sqm[:], xbm[:], xbm[:])
            nc.tensor.matmul(mstats[0:1, :], ones_bf[:], xbm[:],
                             start=(ci == 0), stop=(ci == NP - 1),
                             skip_group_check=True)
            nc.tensor.matmul(mstats[32:33, :], ones_bf[:], sqm[:],
                             start=(ci == 0), stop=(ci == NP - 1),
                             skip_group_check=True)
            xbms.append(xbm)
        a_row = g_small.tile([1, 512], BF, tag="a_row", name="a_row")
        c_row = g_small.tile([1, 512], BF, tag="c_row", name="c_row")
        _ln_smalls(nc, g_small, stats, 512, eps_c, a_row, c_row)
        a_bc = g_bc.tile([128, 512], BF, tag="a_bc", name="a_bc")
        c_bc = g_bc.tile([128, 512], BF, tag="c_bc", name="c_bc")
        nc.gpsimd.partition_broadcast(a_bc[:], a_row[:])
        nc.gpsimd.partition_broadcast(c_bc[:], c_row[:])
        am_row = g_small.tile([1, 512], BF, tag="a_row", name="am_row")[0:1, 0:256]
        cm_row = g_small.tile([1, 512], BF, tag="c_row", name="cm_row")[0:1, 0:256]
        _ln_smalls(nc, g_small, mstats, 256, eps_c, am_row, cm_row)
        am_bc = g_bc.tile([128, 256], BF, tag="am_bc", name="am_bc")
        cm_bc = g_bc.tile([128, 256], BF, tag="cm_bc", name="cm_bc")
        nc.gpsimd.partition_broadcast(am_bc[:], am_row[:])
        nc.gpsimd.partition_broadcast(cm_bc[:], cm_row[:])
        hg, hmg = [], []
        for ci in range(NP):
            tmp = g_roll.tile([128, 512], BF, tag="h_tmp", name="h_tmp")
            nc.vector.tensor_mul(tmp[:], xbs[ci][:], a_bc[:])
            hgc = g_roll.tile([128, 512], BF, tag=f"h{ci}", name="hgc")
            nc.vector.tensor_add(hgc[:], tmp[:], c_bc[:])
            hg.append(hgc)
            tmpm = g_roll.tile([128, 256], BF, tag="hm_tmp", name="hm_tmp")
            nc.vector.tensor_mul(tmpm[:], xbms[ci][:], am_bc[:])
            hmgc = g_roll.tile([128, 256], BF, tag=f"hm{ci}", name="hmgc")
            nc.vector.tensor_add(hmgc[:], tmpm[:], cm_bc[:])
            hmg.append(hmgc)
        if "h" in dbg:
            for ci in range(NP):
                nc.sync.dma_start(dbg["h"][ci * 128:(ci + 1) * 128, csl], hg[ci][:])
        if "hm" in dbg:
            for ci in range(NP):
                nc.sync.dma_start(dbg["hm"][ci * 128:(ci + 1) * 128, msl], hmg[ci][:])
        return hg, hmg

    def emit_qkv_group(g, hg, hmg):
        """K/Q/V GEMMs for token group g."""
        csl = slice(g * 512, (g + 1) * 512)
        msl = slice(g * 256, (g + 1) * 256)
        for f in range(NP):
            ps = gps.tile([128, 512], FP, tag="ps", name="ps")
            for c in range(NP):
                nc.tensor.matmul(ps[:], w_sb[c][:, C + f * 128:C + (f + 1) * 128],
                                 hg[c][:], start=(c == 0), stop=(c == NP - 1))
            nc.scalar.activation(KT[f][:, csl], ps[:], AF.Identity,
                                 bias=bias_sb[f][:, 1:2])
        for f in range(NP):
            ps = gps.tile([128, 512], FP, tag="ps", name="ps")
            for c in range(NP):
                nc.tensor.matmul(ps[:, 0:256], w_sb[c][:, f * 128:(f + 1) * 128],
                                 hmg[c][:], start=(c == 0), stop=(c == NP - 1))
            nc.scalar.activation(QT[f][:, msl], ps[:, 0:256], AF.Identity,
                                 bias=bias_sb[f][:, 0:1])
        for t in range(4):
            v3 = Vsb[4 * g + t][:].rearrange("p (h d) -> p h d", d=65)
            nc.vector.memset(v3[:, :, 64:65], 1.0)
            for fs in range(2):
                n = 512 if fs == 0 else 256
                nh = n // 64
                ps = gps.tile([128, 512], FP, tag="ps", name="ps")
                for c in range(NP):
                    nc.tensor.matmul(ps[:, 0:n], hg[c][:, t * 128:(t + 1) * 128],
                                     w_sb[c][:, 2 * C + fs * 512:2 * C + fs * 512 + n],
                                     start=(c == 0), stop=(c == NP - 1))
                nc.vector.scalar_tensor_tensor(
                    v3[:, fs * 8:fs * 8 + nh, 0:64],
                    ps[:, 0:n].rearrange("p (h d) -> p h d", d=64),
                    0.0,
                    bv_bc[:, fs * 512:fs * 512 + n].rearrange("p (h d) -> p h d", d=64),
                    AluOpType.add, AluOpType.add)

    def att_head(hh, g, pspool):
        """Causal attention for head hh over query col group g (512 cols).
        Key blocks processed in pairs sharing one [128,1024] PSUM tile so a
        single exp instruction covers both (3D access pattern)."""
        ht, hp = hh // 2, (hh % 2) * 64
        npairs = 4 + 4 * g
        pa = pa_pool.tile([65, 512], FP, tag="pa", name="pa")
        for u in range(npairs):
            sa, sb = 2 * u, 2 * u + 1
            c0 = max(0, u - 4 * g) * 128
            ps = pspool.tile([128, 1024], FP, tag="ps", name="ps")
            qs = QT[ht][hp:hp + 64, g * 512 + c0:(g + 1) * 512]
            nc.tensor.matmul(ps[:, c0:512],
                             KT[ht][hp:hp + 64, sa * 128:(sa + 1) * 128], qs,
                             start=True, stop=True)
            nc.tensor.matmul(ps[:, 512 + c0:1024],
                             KT[ht][hp:hp + 64, sb * 128:(sb + 1) * 128], qs,
                             start=True, stop=True)
            wei = g_wei.tile([128, 1024], BF, tag="wei", name="wei")
            ps3 = ps[:].rearrange("p (two q) -> p two q", two=2)
            we3 = wei[:].rearrange("p (two q) -> p two q", two=2)
            nc.scalar.activation(we3[:, :, c0:512], ps3[:, :, c0:512], AF.Exp,
                                 bias=shift_c[:])
            jd = u - 4 * g
            if 0 <= jd < 4:
                nc.vector.tensor_mul(
                    we3[:, :, jd * 128:(jd + 1) * 128],
                    we3[:, :, jd * 128:(jd + 1) * 128],
                    mask_ab[:].rearrange("p (two q) -> p two q", two=2))
            nc.tensor.matmul(pa[:, c0:512], Vsb[sa][:, hh * 65:(hh + 1) * 65],
                             wei[:, c0:512], start=(u == 0), stop=False,
                             skip_group_check=True)
            nc.tensor.matmul(pa[:, c0:512], Vsb[sb][:, hh * 65:(hh + 1) * 65],
                             wei[:, 512 + c0:1024], start=False,
                             stop=(u == npairs - 1), skip_group_check=True)
        sumrow = g_asc.tile([1, 512], FP, tag="sumrow", name="sumrow")
        nc.vector.tensor_copy(sumrow[:], pa[64:65, :])
        recip = g_asc.tile([1, 512], FP, tag="recip", name="recip")
        scr = g_asc.tile([1, 512], FP, tag="scr", name="scr")
        nc.vector.reciprocal_approx_accurate(recip[:], sumrow[:], scr[:])
        rb = g_rb.tile([64, 512], FP, tag="rb", name="rb")
        nc.gpsimd.partition_broadcast(rb[:], recip[:])
        nc.vector.tensor_mul(attnT[ht][hp:hp + 64, g * 512:(g + 1) * 512],
                             pa[0:64, :], rb[:])

    # phase 1 pipeline with attention group 0 overlapped onto its tail
    cur = {0: emit_ln_group(0)}
    cur[1] = emit_ln_group(1)
    emit_qkv_group(0, *cur[0])
    cur[2] = emit_ln_group(2)
    emit_qkv_group(1, *cur[1])
    for hh in range(H):
        att_head(hh, 0, ps0_pool)
        if hh == 0:
            cur[3] = emit_ln_group(3)
        elif hh == 1:
            emit_qkv_group(2, *cur[2])
        elif hh == 3:
            emit_qkv_group(3, *cur[3])
            for c in range(NP):
                nc.vector.tensor_copy(wproj[c][:], w_sb[c][:, 3 * C:4 * C])

    for c in range(NP):
        if "KT" in dbg:
            nc.sync.dma_start(dbg["KT"][c * 128:(c + 1) * 128, :], KT[c][:])
        if "QT" in dbg:
            nc.sync.dma_start(dbg["QT"][c * 128:(c + 1) * 128, :], QT[c][:])
    if "V" in dbg:
        for t in range(NT):
            nc.sync.dma_start(dbg["V"][t * 128:(t + 1) * 128, :], Vsb[t][:])
    ln_es.close()
    w_es.close()
    ps0_es.close()

    # ========== fused attention(group 1) + proj + LN2 + MLP ==========
    fus_es = ExitStack()
    g_w1 = fus_es.enter_context(tc.tile_pool(name="w1p", bufs=1))
    g_h2 = fus_es.enter_context(tc.tile_pool(name="h2p", bufs=1))
    g_r = fus_es.enter_context(tc.tile_pool(name="rp", bufs=1))
    g_roll2 = fus_es.enter_context(tc.tile_pool(name="mlproll", bufs=2))
    g_bc2 = fus_es.enter_context(tc.tile_pool(name="mlpbc", bufs=1))
    g_small2 = fus_es.enter_context(tc.tile_pool(name="mlpsmall", bufs=1))
    ps1_pool = fus_es.enter_context(tc.tile_pool(name="sps1", bufs=2, space="PSUM"))
    gps3 = fus_es.enter_context(tc.tile_pool(name="mps", bufs=2, space="PSUM"))

    w1_sb = [g_w1.tile([128, 4 * C], BF, tag=f"w1_{c}", name=f"w1_{c}") for c in range(NP)]
    for c in range(NP):
        nc.sync.dma_start(w1_sb[c][:], d_w1[c * 128:(c + 1) * 128, :])

    def emit_proj(g):
        gsl = slice(g * 512, (g + 1) * 512)
        for f in range(NP):
            ps = gps3.tile([128, 512], FP, tag="ps", name="ps")
            for c in range(NP):
                nc.tensor.matmul(ps[:], wproj[c][:, f * 128:(f + 1) * 128],
                                 attnT[c][:, gsl],
                                 start=(c == 0), stop=(c == NP - 1))
            xr = g_roll2.tile([128, 512], FP, tag="xr", name="xr")
            nc.sync.dma_start(xr[:], d_xTm[f * 128:(f + 1) * 128, gsl])
            nc.vector.scalar_tensor_tensor(
                xmid[f][:, gsl], ps[:], bias_sb[f][:, 3:4],
                xr[:], AluOpType.add, AluOpType.add)

    h2 = {}

    def emit_ln2(g):
        gsl = slice(g * 512, (g + 1) * 512)
        stats = gps3.tile([33, 512], FP, tag="st2", name="st2", bufs=1)
        for ci in range(NP):
            sq = g_roll2.tile([128, 512], BF, tag="sq2", name="sq2")
            nc.vector.tensor_mul(sq[:], xmid[ci][:, gsl], xmid[ci][:, gsl])
            nc.tensor.matmul(stats[0:1, :], ones_bf[:], xmid[ci][:, gsl],
                             start=(ci == 0), stop=(ci == NP - 1),
                             skip_group_check=True)
            nc.tensor.matmul(stats[32:33, :], ones_bf[:], sq[:],
                             start=(ci == 0), stop=(ci == NP - 1),
                             skip_group_check=True)
        a_row2 = g_small2.tile([1, 512], BF, tag="a_row2", name="a_row2")
        c_row2 = g_small2.tile([1, 512], BF, tag="c_row2", name="c_row2")
        _ln_smalls(nc, g_small2, stats, 512, eps_c, a_row2, c_row2)
        a_bc2 = g_bc2.tile([128, 512], BF, tag="a2bc", name="a2bc")
        c_bc2 = g_bc2.tile([128, 512], BF, tag="c2bc", name="c2bc")
        nc.gpsimd.partition_broadcast(a_bc2[:], a_row2[:])
        nc.gpsimd.partition_broadcast(c_bc2[:], c_row2[:])
        hts = []
        for ci in range(NP):
            tmp = g_roll2.tile([128, 512], BF, tag="h2tmp", name="h2tmp")
            nc.vector.tensor_mul(tmp[:], xmid[ci][:, gsl], a_bc2[:])
            hh2 = g_h2.tile([128, 512], BF, tag=f"h2_{ci}", name=f"h2_{ci}")
            nc.vector.tensor_add(hh2[:], tmp[:], c_bc2[:])
            hts.append(hh2)
        h2[g] = hts

    r_tiles = {}

    def emit_fc1(g, m0, m1):
        for m in range(m0, m1):
            ps = gps3.tile([128, 512], FP, tag="ps", name="ps")
            for c in range(NP):
                nc.tensor.matmul(ps[:], w1_sb[c][:, m * 128:(m + 1) * 128],
                                 h2[g][c][:],
                                 start=(c == 0), stop=(c == NP - 1))
            r = g_r.tile([128, 512], BF, tag=f"r{m}", name=f"r{m}")
            nc.vector.tensor_scalar(r[:], ps[:],
                                    bias_sb[m % 6][:, 5 + m // 6:6 + m // 6],
                                    0.0, AluOpType.add, AluOpType.max)
            r_tiles[g, m] = r

    w2k = [None] * 6
    w2q = [None] * 6

    def emit_w2_chunk(i):
        """Load w2 columns into the KT[i]/QT[i] buffers (pool-tag reuse) as
        soon as heads 2i/2i+1 have finished reading them."""
        w2k[i] = g_kqv.tile([128, 2048], BF, tag=f"KT{i}", name=f"w2k{i}")
        g0, g1 = i * 2048, (i + 1) * 2048
        st = g0
        while st < g1:
            c = st // 3072
            en = min(g1, (c + 1) * 3072)
            nc.sync.dma_start(w2k[i][:, st - g0:en - g0],
                              d_w2[c * 128:(c + 1) * 128,
                                   st - c * 3072:en - c * 3072])
            st = en
        w2q[i] = g_kqv.tile([128, 1024], BF, tag=f"QT{i}", name=f"w2q{i}")
        q0 = 12288 + 1024 * i
        c = q0 // 3072
        nc.sync.dma_start(w2q[i][:], d_w2[c * 128:(c + 1) * 128,
                                          q0 - c * 3072:q0 - c * 3072 + 1024])

    def w2_slice(m, f):
        g0 = (m // 4) * 3072 + (m % 4) * 768 + f * 128
        if g0 < 12288:
            return w2k[g0 // 2048][:, g0 % 2048:g0 % 2048 + 128]
        rr = g0 - 12288
        return w2q[rr // 1024][:, rr % 1024:rr % 1024 + 128]

    def emit_fc2(g):
        gsl = slice(g * 512, (g + 1) * 512)
        for f in range(NP):
            ps = gps3.tile([128, 512], FP, tag="ps", name="ps")
            for m in range(24):
                nc.tensor.matmul(ps[:], w2_slice(m, f), r_tiles[g, m][:],
                                 start=(m == 0), stop=(m == 23))
            ot = g_roll2.tile([128, 512], FP, tag="ot", name="ot")
            nc.vector.scalar_tensor_tensor(ot[:], ps[:], bias_sb[f][:, 4:5],
                                           xmid[f][:, gsl],
                                           AluOpType.add, AluOpType.add)
            nc.sync.dma_start(d_out[f * 128:(f + 1) * 128, gsl], ot[:])

    # attention group 1 interleaved with proj/LN2/fc1 of group 0 (PE keeps
    # dense work and full clock while the exp chain runs) and with w2 loads
    work = [lambda: emit_proj(0), lambda: emit_ln2(0),
            lambda: emit_fc1(0, 0, 6), lambda: emit_fc1(0, 6, 12),
            lambda: emit_fc1(0, 12, 18), lambda: emit_fc1(0, 18, 24)]
    wi = 0
    for hh in range(H):
        att_head(hh, 1, ps1_pool)
        if hh % 2 == 1:
            emit_w2_chunk(hh // 2)
        if hh >= 2 and wi < len(work):
            work[wi]()
            wi += 1
    while wi < len(work):
        work[wi]()
        wi += 1
    if "attnT" in dbg:
        for c in range(NP):
            nc.sync.dma_start(dbg["attnT"][c * 128:(c + 1) * 128, :], attnT[c][:])

    emit_proj(1)
    emit_ln2(1)
    emit_fc2(0)
    emit_fc1(1, 0, 24)
    emit_fc2(1)
    if "xmid" in dbg:
        for c in range(NP):
            nc.sync.dma_start(dbg["xmid"][c * 128:(c + 1) * 128, :], xmid[c][:])
    fus_es.close()
    fusA_es.close()
    attnT_es.close()
    kqv_stack.close()
    es.close()


# ---------------------------------------------------------------------------
# host side
# ---------------------------------------------------------------------------

def _mycols(half):
    blocks = np.arange(8) * 2 + half
    return (blocks[:, None] * 128 + np.arange(128)[None, :]).reshape(-1)


def _prep_inputs(x, wq, bq, wk, bk, wv, bv, w_proj, b_proj, w1, b1, w2, b2,
                 g1, beta1, g2, beta2):
    x = np.asarray(x, np.float32)
    wq_f = np.ascontiguousarray(np.transpose(np.asarray(wq, np.float32), (1, 0, 2)).reshape(C, C))
    wk_f = np.ascontiguousarray(np.transpose(np.asarray(wk, np.float32), (1, 0, 2)).reshape(C, C))
    wv_f = np.ascontiguousarray(np.transpose(np.asarray(wv, np.float32), (1, 0, 2)).reshape(C, C))
    g1 = np.asarray(g1, np.float32); beta1 = np.asarray(beta1, np.float32)
    g2 = np.asarray(g2, np.float32); beta2 = np.asarray(beta2, np.float32)
    w1 = np.asarray(w1, np.float32); w2 = np.asarray(w2, np.float32)
    w_proj = np.asarray(w_proj, np.float32)

    wq_g = g1[:, None] * wq_f
    wk_g = g1[:, None] * wk_f
    wv_g = g1[:, None] * wv_f
    bq_f = beta1 @ wq_f + np.asarray(bq, np.float32).reshape(-1)
    bk_f = beta1 @ wk_f + np.asarray(bk, np.float32).reshape(-1)
    bv_f = beta1 @ wv_f + np.asarray(bv, np.float32).reshape(-1)
    w1_g = g2[:, None] * w1
    b1_f = beta2 @ w1 + np.asarray(b1, np.float32)

    wqkvp = np.concatenate([wq_g, wk_g, wv_g, w_proj], axis=1).astype(bf16)
    w1p = w1_g.astype(bf16)
    w2p = np.ascontiguousarray(
        w2.reshape(6, 4, 128, C).transpose(0, 2, 1, 3).reshape(C, 4 * C)).astype(bf16)

    biasp = np.zeros((C, 9), np.float32)
    biasp[:, 0] = bq_f
    biasp[:, 1] = bk_f
    biasp[:, 2] = bv_f
    biasp[:, 3] = np.asarray(b_proj, np.float32)
    biasp[:, 4] = np.asarray(b2, np.float32)
    biasp[:, 5:9] = b1_f.reshape(4, C).T
    bvrow = bv_f.reshape(1, C).astype(np.float32)

    tri = np.tril(np.ones((128, 128), np.float32)).T  # [s, q]: 1 iff s <= q
    in_maps = []
    for core in range(8):
        b, half = core // 2, core % 2
        xT = np.ascontiguousarray(x[b].T)
        xTm = np.ascontiguousarray(xT[:, _mycols(half)])
        masks = np.zeros((256, 128), np.float32)
        if half == 0:
            masks[0:128] = tri
            masks[128:256] = 0.0
        else:
            masks[0:128] = 1.0
            masks[128:256] = tri
        in_maps.append({
            "xT": xT, "xTm": xTm,
            "wqkvp": wqkvp, "w1p": w1p, "w2p": w2p,
            "biasp": biasp, "bvrow": bvrow,
            "masks": masks.astype(bf16),
        })
    return in_maps


def _assemble(results, dtype):
    out = np.empty((B, T, C), dtype)
    for core in range(8):
        b, half = core // 2, core % 2
        out[b, _mycols(half), :] = results[core]["outT"].T
    return out


def kernel(**inputs):
    in_maps = _prep_inputs(**inputs)
    if "nc" not in _cache:
        _cache["nc"] = _build()
    res = bass_utils.run_bass_kernel_spmd(_cache["nc"], in_maps,
                                          core_ids=list(range(8)))
    return _assemble(res.results, np.asarray(inputs["x"]).dtype)


# revision 43
# speedup vs baseline: 1.1402x; 1.1113x over previous
"""Transformer block (LN -> 12-head causal attention -> residual -> LN -> MLP
-> residual) for B=4, T=2048, C=768 on 8 trn2 NeuronCores.

Sharding: core = (batch, token-half). Each core handles one batch's K/V in
full and produces the final output for half the tokens (even or odd 128-token
blocks, which balances the causal-attention triangle). No collectives; all
per-core structural differences are carried in input *data* (host-gathered
xTm, causal-boundary mask tiles) so a single SPMD program runs on all 8
cores.

On-chip layout is feature-major ("transposed", [C, T]). LN statistics are
computed with ones-vector matmuls on the tensor engine and the LN1+QKV
pipeline is interleaved per 512-token group so the PE never starves.
Attention processes all 1024 owned query columns per head in one pass
(1024-wide exp tiles halve the scalar-engine instruction count); softmax
row-sums ride a ones column appended to V and are inverted with the fast
Newton-Raphson reciprocal.
"""

import math
import os
import sys

for _p in ("/opt/trn_rl_repo", "/root/.axon_site/_ro/trn_rl_repo"):
    if os.path.isdir(_p) and _p not in sys.path:
        sys.path.append(_p)

import numpy as np
import ml_dtypes

import concourse.bacc as bacc
import concourse.tile as tile
import concourse.mybir as mybir
from concourse import bass_utils
from concourse.alu_op_type import AluOpType
from concourse.tile_rust import add_dep_helper

BF = mybir.dt.bfloat16
FP = mybir.dt.float32
AF = mybir.ActivationFunctionType

B, T, C, H, HD = 4, 2048, 768, 12, 64
EPS = 1e-5
SHIFT = 40.0  # constant softmax shift: exp(s - SHIFT); exact softmax
NP = C // 128  # 6 feature partition-tiles
NT = T // 128  # 16 token blocks
TM = T // 2    # 1024 tokens owned per core
NG = 4         # 512-token groups
bf16 = ml_dtypes.bfloat16

_cache = {}


def _build(debug=False):
    nc = bacc.Bacc("TRN2", target_bir_lowering=False, debug=False)
    d_xT = nc.dram_tensor("xT", [C, T], FP, kind="ExternalInput").ap()
    d_xTm = nc.dram_tensor("xTm", [C, TM], FP, kind="ExternalInput").ap()
    d_wqkvp = nc.dram_tensor("wqkvp", [C, 4 * C], BF, kind="ExternalInput").ap()
    d_w1 = nc.dram_tensor("w1p", [C, 4 * C], BF, kind="ExternalInput").ap()
    d_w2 = nc.dram_tensor("w2p", [C, 4 * C], BF, kind="ExternalInput").ap()
    d_bias = nc.dram_tensor("biasp", [C, 9], FP, kind="ExternalInput").ap()
    d_bvrow = nc.dram_tensor("bvrow", [1, C], FP, kind="ExternalInput").ap()
    d_masks = nc.dram_tensor("masks", [256, 128], BF, kind="ExternalInput").ap()
    d_out = nc.dram_tensor("outT", [C, TM], FP, kind="ExternalOutput").ap()
    if debug is True:
        debug = ["h", "hm", "KT", "QT", "V", "attnT", "xmid"]
    debug = debug or []
    dbg = {}
    if "h" in debug:
        dbg["h"] = nc.dram_tensor("dbg_h", [C, T], BF, kind="ExternalOutput").ap()
    if "hm" in debug:
        dbg["hm"] = nc.dram_tensor("dbg_hm", [C, TM], BF, kind="ExternalOutput").ap()
    if "KT" in debug:
        dbg["KT"] = nc.dram_tensor("dbg_KT", [C, T], BF, kind="ExternalOutput").ap()
    if "QT" in debug:
        dbg["QT"] = nc.dram_tensor("dbg_QT", [C, TM], BF, kind="ExternalOutput").ap()
    if "V" in debug:
        dbg["V"] = nc.dram_tensor("dbg_V", [T, H * 65], BF, kind="ExternalOutput").ap()
    if "attnT" in debug:
        dbg["attnT"] = nc.dram_tensor("dbg_attnT", [C, TM], BF, kind="ExternalOutput").ap()
    if "xmid" in debug:
        dbg["xmid"] = nc.dram_tensor("dbg_xmid", [C, TM], FP, kind="ExternalOutput").ap()

    with tile.TileContext(nc) as tc:
        _body(nc, tc, d_xT, d_xTm, d_wqkvp, d_w1, d_w2, d_bias, d_bvrow,
              d_masks, d_out, dbg)
    nc.compile()
    return nc


def _ln_smalls(nc, small, stats, ncols, eps_c, a_dst, c_dst):
    """From accumulated [33, ncols] stats (row 0 = sum, row 32 = sumsq),
    produce bf16 [1, ncols] rows a5b (1/std) and c5b (-mu/std)."""
    n = ncols
    mu = small.tile([1, 512], FP, tag="mu", name="mu")[0:1, 0:n]
    nc.scalar.mul(mu, stats[0:1, :], 1.0 / C)
    m2 = small.tile([1, 512], FP, tag="m2", name="m2")[0:1, 0:n]
    nc.scalar.mul(m2, stats[32:33, :], 1.0 / C)
    var = small.tile([1, 512], FP, tag="va", name="va")[0:1, 0:n]
    nc.vector.tensor_mul(var, mu, mu)
    nc.vector.tensor_sub(var, m2, var)
    std = small.tile([1, 512], FP, tag="sd", name="sd")[0:1, 0:n]
    nc.scalar.activation(std, var, AF.Sqrt, bias=eps_c[0:1, 0:1])
    a5 = small.tile([1, 512], FP, tag="a5", name="a5")[0:1, 0:n]
    nc.vector.reciprocal_approx_accurate(a5, std, var)
    c5 = small.tile([1, 512], FP, tag="c5", name="c5")[0:1, 0:n]
    nc.vector.scalar_tensor_tensor(c5, mu, -1.0, a5,
                                   AluOpType.mult, AluOpType.mult)
    nc.vector.tensor_copy(a_dst[:], a5)
    nc.vector.tensor_copy(c_dst[:], c5)


def _body(nc, tc, d_xT, d_xTm, d_wqkvp, d_w1, d_w2, d_bias, d_bvrow,
          d_masks, d_out, dbg={}):
    from contextlib import ExitStack

    es = ExitStack()
    g_const = es.enter_context(tc.tile_pool(name="const", bufs=1))
    g_xmid = es.enter_context(tc.tile_pool(name="xmid", bufs=1))
    # bf16 residual trunk: halves SBUF so MLP weights can coexist with K/Q/V
    xmid = [g_xmid.tile([128, TM], BF, tag=f"xm{i}", name=f"xm{i}") for i in range(NP)]
    kqv_stack = ExitStack()
    w_es = ExitStack()
    g_kqv = kqv_stack.enter_context(tc.tile_pool(name="kqv", bufs=1))

    # ---- constants ----
    ones_bf = g_const.tile([128, 1], BF, tag="ones_bf", name="ones_bf")
    nc.vector.memset(ones_bf[:], 1.0)
    eps_c = g_const.tile([128, 1], FP, tag="eps_c", name="eps_c")
    nc.vector.memset(eps_c[:], EPS)
    shift_c = g_const.tile([128, 1], FP, tag="shift_c", name="shift_c")
    nc.vector.memset(shift_c[:], -SHIFT)
    bias_sb = [g_const.tile([128, 9], FP, tag=f"bias{f}", name=f"bias{f}") for f in range(NP)]
    for f in range(NP):
        nc.sync.dma_start(bias_sb[f][:], d_bias[f * 128:(f + 1) * 128, :])
    mask_ab = g_const.tile([128, 256], BF, tag="mask_ab", name="mask_ab")
    nc.sync.dma_start(mask_ab[:, 0:128], d_masks[0:128, :])
    nc.sync.dma_start(mask_ab[:, 128:256], d_masks[128:256, :])
    bv_row = g_const.tile([1, C], FP, tag="bv_row", name="bv_row")
    nc.sync.dma_start(bv_row[:], d_bvrow[:])
    bv_rb = g_const.tile([1, C], BF, tag="bv_rb", name="bv_rb")
    nc.vector.tensor_copy(bv_rb[:], bv_row[:])
    bv_bc = g_const.tile([128, C], BF, tag="bv_bc", name="bv_bc")
    nc.gpsimd.partition_broadcast(bv_bc[:], bv_rb[:])

    # ---- persistent activation storage ----
    KT = [g_kqv.tile([128, T], BF, tag=f"KT{i}", name=f"KT{i}") for i in range(NP)]
    QT = [g_kqv.tile([128, TM], BF, tag=f"QT{i}", name=f"QT{i}") for i in range(NP)]
    Vsb = [g_kqv.tile([128, H * 65], BF, tag=f"V{t}", name=f"V{t}") for t in range(NT)]
    wproj = [g_kqv.tile([128, C], BF, tag=f"wp{c}", name=f"wp{c}") for c in range(NP)]

    # ---- pools created up-front in global LIFO order ----
    attnT_es = ExitStack()
    g_attnT = attnT_es.enter_context(tc.tile_pool(name="attnT", bufs=1))
    attnT = [g_attnT.tile([128, TM], BF, tag=f"aT{i}", name=f"aT{i}") for i in range(NP)]

    fusA_es = ExitStack()
    g_wei = fusA_es.enter_context(tc.tile_pool(name="wei", bufs=2))
    g_asc = fusA_es.enter_context(tc.tile_pool(name="ascratch", bufs=1))
    g_rb = fusA_es.enter_context(tc.tile_pool(name="rbpool", bufs=1))
    pa_pool = fusA_es.enter_context(tc.tile_pool(name="aps", bufs=2, space="PSUM"))
    ps0_es = ExitStack()
    ps0_pool = ps0_es.enter_context(tc.tile_pool(name="sps0", bufs=1, space="PSUM"))

    # ---- weights for attention part (pool above att pools so it can close
    # right after the QKV phase) ----
    g_w = w_es.enter_context(tc.tile_pool(name="wqkvp", bufs=1))
    w_sb = [g_w.tile([128, 4 * C], BF, tag=f"w{c}", name=f"w{c}") for c in range(NP)]

    # ================= LN1 + QKV, pipelined per 512-token group =============
    ln_es = ExitStack()
    g_roll = ln_es.enter_context(tc.tile_pool(name="lnroll", bufs=2))
    g_bc = ln_es.enter_context(tc.tile_pool(name="lnbc", bufs=2))
    g_small = ln_es.enter_context(tc.tile_pool(name="lnsmall", bufs=1))
    sps = ln_es.enter_context(tc.tile_pool(name="statps", bufs=1, space="PSUM"))
    gps = ln_es.enter_context(tc.tile_pool(name="gemmps", bufs=2, space="PSUM"))

    def emit_ln_group(g):
        """LN1 stats + h/hm for token group g (cols g*512..(g+1)*512 of T,
        owned cols g*256..(g+1)*256 of TM). Returns per-group h/hm tiles."""
        csl = slice(g * 512, (g + 1) * 512)
        msl = slice(g * 256, (g + 1) * 256)
        stats = sps.tile([33, 512], FP, tag="stf", name="stf")
        mstats = sps.tile([33, 256], FP, tag="stm", name="stm")
        xbs, xbms = [], []
        for ci in range(NP):
            xt = g_roll.tile([128, 512], FP, tag="xr", name="xr")
            nc.sync.dma_start(xt[:], d_xT[ci * 128:(ci + 1) * 128, csl])
            xb = g_roll.tile([128, 512], BF, tag="xb", name="xb", bufs=6)
            nc.vector.tensor_copy(xb[:], xt[:])
            sq = g_roll.tile([128, 512], BF, tag="sq", name="sq")
            nc.vector.tensor_mul(sq[:], xb[:], xb[:])
            nc.tensor.matmul(stats[0:1, :], ones_bf[:], xb[:],
                             start=(ci == 0), stop=(ci == NP - 1),
                             skip_group_check=True)
            nc.tensor.matmul(stats[32:33, :], ones_bf[:], sq[:],
                             start=(ci == 0), stop=(ci == NP - 1),
                             skip_group_check=True)
            xbs.append(xb)
            xtm = g_roll.tile([128, 256], FP, tag="xrm", name="xrm")
            nc.sync.dma_start(xtm[:], d_xTm[ci * 128:(ci + 1) * 128, msl])
            xbm = g_roll.tile([128, 256], BF, tag="xbm", name="xbm", bufs=6)
            nc.vector.tensor_copy(xbm[:], xtm[:])
            sqm = g_roll.tile([128, 256], BF, tag="sqm", name="sqm")
            nc.vector.tensor_mul(sqm[:], xbm[:], xbm[:])
            nc.tensor.matmul(mstats[0:1, :], ones_bf[:], xbm[:],
                             start=(ci == 0), stop=(ci == NP - 1),
                             skip_group_check=True)
            nc.tensor.matmul(mstats[32:33, :], ones_bf[:], sqm[:],
                             start=(ci == 0), stop=(ci == NP - 1),
                             skip_group_check=True)
            xbms.append(xbm)
        a_row = g_small.tile([1, 512], BF, tag="a_row", name="a_row")
        c_row = g_small.tile([1, 512], BF, tag="c_row", name="c_row")
        _ln_smalls(nc, g_small, stats, 512, eps_c, a_row, c_row)
        a_bc = g_bc.tile([128, 512], BF, tag="a_bc", name="a_bc")
        c_bc = g_bc.tile([128, 512], BF, tag="c_bc", name="c_bc")
        nc.gpsimd.partition_broadcast(a_bc[:], a_row[:])
        nc.gpsimd.partition_broadcast(c_bc[:], c_row[:])
        am_row = g_small.tile([1, 512], BF, tag="a_row", name="am_row")[0:1, 0:256]
        cm_row = g_small.tile([1, 512], BF, tag="c_row", name="cm_row")[0:1, 0:256]
        _ln_smalls(nc, g_small, mstats, 256, eps_c, am_row, cm_row)
        am_bc = g_bc.tile([128, 256], BF, tag="am_bc", name="am_bc")
        cm_bc = g_bc.tile([128, 256], BF, tag="cm_bc", name="cm_bc")
        nc.gpsimd.partition_broadcast(am_bc[:], am_row[:])
        nc.gpsimd.partition_broadcast(cm_bc[:], cm_row[:])
        hg, hmg = [], []
        for ci in range(NP):
            tmp = g_roll.tile([128, 512], BF, tag="h_tmp", name="h_tmp")
            nc.vector.tensor_mul(tmp[:], xbs[ci][:], a_bc[:])
            hgc = g_roll.tile([128, 512], BF, tag=f"h{ci}", name="hgc")
            nc.vector.tensor_add(hgc[:], tmp[:], c_bc[:])
            hg.append(hgc)
            tmpm = g_roll.tile([128, 256], BF, tag="hm_tmp", name="hm_tmp")
            nc.vector.tensor_mul(tmpm[:], xbms[ci][:], am_bc[:])
            hmgc = g_roll.tile([128, 256], BF, tag=f"hm{ci}", name="hmgc")
            nc.vector.tensor_add(hmgc[:], tmpm[:], cm_bc[:])
            hmg.append(hmgc)
        if "h" in dbg:
            for ci in range(NP):
                nc.sync.dma_start(dbg["h"][ci * 128:(ci + 1) * 128, csl], hg[ci][:])
        if "hm" in dbg:
            for ci in range(NP):
                nc.sync.dma_start(dbg["hm"][ci * 128:(ci + 1) * 128, msl], hmg[ci][:])
        return hg, hmg

    def emit_qkv_group(g, hg, hmg):
        """K/Q/V GEMMs for token group g."""
        csl = slice(g * 512, (g + 1) * 512)
        msl = slice(g * 256, (g + 1) * 256)
        for f in range(NP):
            ps = gps.tile([128, 512], FP, tag="ps", name="ps")
            for c in range(NP):
                nc.tensor.matmul(ps[:], w_sb[c][:, C + f * 128:C + (f + 1) * 128],
                                 hg[c][:], start=(c == 0), stop=(c == NP - 1))
            nc.scalar.activation(KT[f][:, csl], ps[:], AF.Identity,
                                 bias=bias_sb[f][:, 1:2])
        for f in range(NP):
            ps = gps.tile([128, 512], FP, tag="ps", name="ps")
            for c in range(NP):
                nc.tensor.matmul(ps[:, 0:256], w_sb[c][:, f * 128:(f + 1) * 128],
                                 hmg[c][:], start=(c == 0), stop=(c == NP - 1))
            nc.scalar.activation(QT[f][:, msl], ps[:, 0:256], AF.Identity,
                                 bias=bias_sb[f][:, 0:1])
        for t in range(4):
            v3 = Vsb[4 * g + t][:].rearrange("p (h d) -> p h d", d=65)
            nc.vector.memset(v3[:, :, 64:65], 1.0)
            for fs in range(2):
                n = 512 if fs == 0 else 256
                nh = n // 64
                ps = gps.tile([128, 512], FP, tag="ps", name="ps")
                for c in range(NP):
                    nc.tensor.matmul(ps[:, 0:n], hg[c][:, t * 128:(t + 1) * 128],
                                     w_sb[c][:, 2 * C + fs * 512:2 * C + fs * 512 + n],
                                     start=(c == 0), stop=(c == NP - 1))
                nc.vector.scalar_tensor_tensor(
                    v3[:, fs * 8:fs * 8 + nh, 0:64],
                    ps[:, 0:n].rearrange("p (h d) -> p h d", d=64),
                    0.0,
                    bv_bc[:, fs * 512:fs * 512 + n].rearrange("p (h d) -> p h d", d=64),
                    AluOpType.add, AluOpType.add)

    def att_head(hh, g, pspool):
        """Causal attention for head hh over query col group g (512 cols).
        Key blocks processed in pairs sharing one [128,1024] PSUM tile so a
        single exp instruction covers both (3D access pattern)."""
        ht, hp = hh // 2, (hh % 2) * 64
        npairs = 4 + 4 * g
        pa = pa_pool.tile([65, 512], FP, tag="pa", name="pa")
        for u in range(npairs):
            sa, sb = 2 * u, 2 * u + 1
            c0 = max(0, u - 4 * g) * 128
            ps = pspool.tile([128, 1024], FP, tag="ps", name="ps")
            qs = QT[ht][hp:hp + 64, g * 512 + c0:(g + 1) * 512]
            nc.tensor.matmul(ps[:, c0:512],
                             KT[ht][hp:hp + 64, sa * 128:(sa + 1) * 128], qs,
                             start=True, stop=True)
            nc.tensor.matmul(ps[:, 512 + c0:1024],
                             KT[ht][hp:hp + 64, sb * 128:(sb + 1) * 128], qs,
                             start=True, stop=True)
            wei = g_wei.tile([128, 1024], BF, tag="wei", name="wei")
            ps3 = ps[:].rearrange("p (two q) -> p two q", two=2)
            we3 = wei[:].rearrange("p (two q) -> p two q", two=2)
            nc.scalar.activation(we3[:, :, c0:512], ps3[:, :, c0:512], AF.Exp,
                                 bias=shift_c[:])
            jd = u - 4 * g
            if 0 <= jd < 4:
                nc.vector.tensor_mul(
                    we3[:, :, jd * 128:(jd + 1) * 128],
                    we3[:, :, jd * 128:(jd + 1) * 128],
                    mask_ab[:].rearrange("p (two q) -> p two q", two=2))
            nc.tensor.matmul(pa[:, c0:512], Vsb[sa][:, hh * 65:(hh + 1) * 65],
                             wei[:, c0:512], start=(u == 0), stop=False,
                             skip_group_check=True)
            nc.tensor.matmul(pa[:, c0:512], Vsb[sb][:, hh * 65:(hh + 1) * 65],
                             wei[:, 512 + c0:1024], start=False,
                             stop=(u == npairs - 1), skip_group_check=True)
        sumrow = g_asc.tile([1, 512], FP, tag="sumrow", name="sumrow")
        nc.vector.tensor_copy(sumrow[:], pa[64:65, :])
        recip = g_asc.tile([1, 512], FP, tag="recip", name="recip")
        scr = g_asc.tile([1, 512], FP, tag="scr", name="scr")
        nc.vector.reciprocal_approx_accurate(recip[:], sumrow[:], scr[:])
        rb = g_rb.tile([64, 512], FP, tag="rb", name="rb")
        nc.gpsimd.partition_broadcast(rb[:], recip[:])
        nc.vector.tensor_mul(attnT[ht][hp:hp + 64, g * 512:(g + 1) * 512],
                             pa[0:64, :], rb[:])

    # phase 1 pipeline with attention group 0 overlapped onto its tail
    cur = {0: emit_ln_group(0)}
    cur[1] = emit_ln_group(1)
    for c in range(NP):
        nc.sync.dma_start(w_sb[c][:], d_wqkvp[c * 128:(c + 1) * 128, :])
    emit_qkv_group(0, *cur[0])
    cur[2] = emit_ln_group(2)
    emit_qkv_group(1, *cur[1])
    for hh in range(H):
        att_head(hh, 0, ps0_pool)
        if hh == 0:
            cur[3] = emit_ln_group(3)
        elif hh == 2:
            emit_qkv_group(2, *cur[2])
        elif hh == 6:
            emit_qkv_group(3, *cur[3])
            for c in range(NP):
                nc.vector.tensor_copy(wproj[c][:], w_sb[c][:, 3 * C:4 * C])

    for c in range(NP):
        if "KT" in dbg:
            nc.sync.dma_start(dbg["KT"][c * 128:(c + 1) * 128, :], KT[c][:])
        if "QT" in dbg:
            nc.sync.dma_start(dbg["QT"][c * 128:(c + 1) * 128, :], QT[c][:])
    if "V" in dbg:
        for t in range(NT):
            nc.sync.dma_start(dbg["V"][t * 128:(t + 1) * 128, :], Vsb[t][:])
    ln_es.close()
    w_es.close()
    ps0_es.close()

    # ========== fused attention(group 1) + proj + LN2 + MLP ==========
    fus_es = ExitStack()
    g_w1 = fus_es.enter_context(tc.tile_pool(name="w1p", bufs=1))
    g_h2 = fus_es.enter_context(tc.tile_pool(name="h2p", bufs=1))
    g_r = fus_es.enter_context(tc.tile_pool(name="rp", bufs=1))
    g_roll2 = fus_es.enter_context(tc.tile_pool(name="mlproll", bufs=2))
    g_bc2 = fus_es.enter_context(tc.tile_pool(name="mlpbc", bufs=1))
    g_small2 = fus_es.enter_context(tc.tile_pool(name="mlpsmall", bufs=1))
    ps1_pool = fus_es.enter_context(tc.tile_pool(name="sps1", bufs=2, space="PSUM"))
    gps3 = fus_es.enter_context(tc.tile_pool(name="mps", bufs=2, space="PSUM"))

    w1_sb = [g_w1.tile([128, 4 * C], BF, tag=f"w1_{c}", name=f"w1_{c}") for c in range(NP)]
    for c in range(NP):
        nc.sync.dma_start(w1_sb[c][:], d_w1[c * 128:(c + 1) * 128, :])

    def emit_proj(g):
        gsl = slice(g * 512, (g + 1) * 512)
        for f in range(NP):
            ps = gps3.tile([128, 512], FP, tag="ps", name="ps")
            for c in range(NP):
                nc.tensor.matmul(ps[:], wproj[c][:, f * 128:(f + 1) * 128],
                                 attnT[c][:, gsl],
                                 start=(c == 0), stop=(c == NP - 1))
            xr = g_roll2.tile([128, 512], FP, tag="xr", name="xr")
            nc.sync.dma_start(xr[:], d_xTm[f * 128:(f + 1) * 128, gsl])
            nc.vector.scalar_tensor_tensor(
                xmid[f][:, gsl], ps[:], bias_sb[f][:, 3:4],
                xr[:], AluOpType.add, AluOpType.add)

    h2 = {}

    def emit_ln2(g):
        gsl = slice(g * 512, (g + 1) * 512)
        stats = pa_pool.tile([65, 512], FP, tag="pa", name="st2")
        for ci in range(NP):
            sq = g_roll2.tile([128, 512], BF, tag="sq2", name="sq2")
            nc.vector.tensor_mul(sq[:], xmid[ci][:, gsl], xmid[ci][:, gsl])
            nc.tensor.matmul(stats[0:1, :], ones_bf[:], xmid[ci][:, gsl],
                             start=(ci == 0), stop=(ci == NP - 1),
                             skip_group_check=True)
            nc.tensor.matmul(stats[32:33, :], ones_bf[:], sq[:],
                             start=(ci == 0), stop=(ci == NP - 1),
                             skip_group_check=True)
        a_row2 = g_small2.tile([1, 512], BF, tag="a_row2", name="a_row2")
        c_row2 = g_small2.tile([1, 512], BF, tag="c_row2", name="c_row2")
        _ln_smalls(nc, g_small2, stats, 512, eps_c, a_row2, c_row2)
        a_bc2 = g_bc2.tile([128, 512], BF, tag="a2bc", name="a2bc")
        c_bc2 = g_bc2.tile([128, 512], BF, tag="c2bc", name="c2bc")
        nc.gpsimd.partition_broadcast(a_bc2[:], a_row2[:])
        nc.gpsimd.partition_broadcast(c_bc2[:], c_row2[:])
        hts = []
        for ci in range(NP):
            tmp = g_roll2.tile([128, 512], BF, tag="h2tmp", name="h2tmp")
            nc.vector.tensor_mul(tmp[:], xmid[ci][:, gsl], a_bc2[:])
            hh2 = g_h2.tile([128, 512], BF, tag=f"h2_{ci}", name=f"h2_{ci}")
            nc.vector.tensor_add(hh2[:], tmp[:], c_bc2[:])
            hts.append(hh2)
        h2[g] = hts

    r_tiles = {}

    def emit_fc1(g, m0, m1):
        for m in range(m0, m1):
            ps = gps3.tile([128, 512], FP, tag="ps", name="ps")
            for c in range(NP):
                nc.tensor.matmul(ps[:], w1_sb[c][:, m * 128:(m + 1) * 128],
                                 h2[g][c][:],
                                 start=(c == 0), stop=(c == NP - 1))
            r = g_r.tile([128, 512], BF, tag=f"r{m}", name=f"r{m}")
            nc.vector.tensor_scalar(r[:], ps[:],
                                    bias_sb[m % 6][:, 5 + m // 6:6 + m // 6],
                                    0.0, AluOpType.add, AluOpType.max)
            r_tiles[g, m] = r

    w2k = [None] * 6
    w2q = [None] * 6

    def emit_w2_chunk(i):
        """Load w2 columns into the KT[i]/QT[i] buffers (pool-tag reuse) as
        soon as heads 2i/2i+1 have finished reading them."""
        w2k[i] = g_kqv.tile([128, 2048], BF, tag=f"KT{i}", name=f"w2k{i}")
        g0, g1 = i * 2048, (i + 1) * 2048
        st = g0
        while st < g1:
            c = st // 3072
            en = min(g1, (c + 1) * 3072)
            nc.sync.dma_start(w2k[i][:, st - g0:en - g0],
                              d_w2[c * 128:(c + 1) * 128,
                                   st - c * 3072:en - c * 3072])
            st = en
        w2q[i] = g_kqv.tile([128, 1024], BF, tag=f"QT{i}", name=f"w2q{i}")
        q0 = 12288 + 1024 * i
        c = q0 // 3072
        nc.sync.dma_start(w2q[i][:], d_w2[c * 128:(c + 1) * 128,
                                          q0 - c * 3072:q0 - c * 3072 + 1024])

    def w2_slice(m, f):
        g0 = (m // 4) * 3072 + (m % 4) * 768 + f * 128
        if g0 < 12288:
            return w2k[g0 // 2048][:, g0 % 2048:g0 % 2048 + 128]
        rr = g0 - 12288
        return w2q[rr // 1024][:, rr % 1024:rr % 1024 + 128]

    def emit_fc2(g):
        gsl = slice(g * 512, (g + 1) * 512)
        for f in range(NP):
            ps = gps3.tile([128, 512], FP, tag="ps", name="ps")
            for m in range(24):
                nc.tensor.matmul(ps[:], w2_slice(m, f), r_tiles[g, m][:],
                                 start=(m == 0), stop=(m == 23))
            ot = g_roll2.tile([128, 512], FP, tag="ot", name="ot")
            nc.vector.scalar_tensor_tensor(ot[:], ps[:], bias_sb[f][:, 4:5],
                                           xmid[f][:, gsl],
                                           AluOpType.add, AluOpType.add)
            nc.sync.dma_start(d_out[f * 128:(f + 1) * 128, gsl], ot[:])

    # attention group 1 interleaved with proj/LN2/fc1 of group 0 (PE keeps
    # dense work and full clock while the exp chain runs) and with w2 loads
    work = [lambda: emit_proj(0), lambda: emit_ln2(0),
            lambda: emit_fc1(0, 0, 6), lambda: emit_fc1(0, 6, 12),
            lambda: emit_fc1(0, 12, 18), lambda: emit_fc1(0, 18, 24)]
    wi = 0
    for hh in range(H):
        att_head(hh, 1, ps1_pool)
        if hh % 2 == 1:
            emit_w2_chunk(hh // 2)
        if hh >= 2 and wi < len(work):
            work[wi]()
            wi += 1
    while wi < len(work):
        work[wi]()
        wi += 1
    if "attnT" in dbg:
        for c in range(NP):
            nc.sync.dma_start(dbg["attnT"][c * 128:(c + 1) * 128, :], attnT[c][:])

    emit_proj(1)
    emit_ln2(1)
    emit_fc2(0)
    emit_fc1(1, 0, 24)
    emit_fc2(1)
    if "xmid" in dbg:
        for c in range(NP):
            nc.sync.dma_start(dbg["xmid"][c * 128:(c + 1) * 128, :], xmid[c][:])
    fus_es.close()
    fusA_es.close()
    attnT_es.close()
    kqv_stack.close()
    es.close()


# ---------------------------------------------------------------------------
# host side
# ---------------------------------------------------------------------------

def _mycols(half):
    blocks = np.arange(8) * 2 + half
    return (blocks[:, None] * 128 + np.arange(128)[None, :]).reshape(-1)


def _prep_inputs(x, wq, bq, wk, bk, wv, bv, w_proj, b_proj, w1, b1, w2, b2,
                 g1, beta1, g2, beta2):
    x = np.asarray(x, np.float32)
    wq_f = np.ascontiguousarray(np.transpose(np.asarray(wq, np.float32), (1, 0, 2)).reshape(C, C))
    wk_f = np.ascontiguousarray(np.transpose(np.asarray(wk, np.float32), (1, 0, 2)).reshape(C, C))
    wv_f = np.ascontiguousarray(np.transpose(np.asarray(wv, np.float32), (1, 0, 2)).reshape(C, C))
    g1 = np.asarray(g1, np.float32); beta1 = np.asarray(beta1, np.float32)
    g2 = np.asarray(g2, np.float32); beta2 = np.asarray(beta2, np.float32)
    w1 = np.asarray(w1, np.float32); w2 = np.asarray(w2, np.float32)
    w_proj = np.asarray(w_proj, np.float32)

    wq_g = g1[:, None] * wq_f
    wk_g = g1[:, None] * wk_f
    wv_g = g1[:, None] * wv_f
    bq_f = beta1 @ wq_f + np.asarray(bq, np.float32).reshape(-1)
    bk_f = beta1 @ wk_f + np.asarray(bk, np.float32).reshape(-1)
    bv_f = beta1 @ wv_f + np.asarray(bv, np.float32).reshape(-1)
    w1_g = g2[:, None] * w1
    b1_f = beta2 @ w1 + np.asarray(b1, np.float32)

    wqkvp = np.concatenate([wq_g, wk_g, wv_g, w_proj], axis=1).astype(bf16)
    w1p = w1_g.astype(bf16)
    w2p = np.ascontiguousarray(
        w2.reshape(6, 4, 128, C).transpose(0, 2, 1, 3).reshape(C, 4 * C)).astype(bf16)

    biasp = np.zeros((C, 9), np.float32)
    biasp[:, 0] = bq_f
    biasp[:, 1] = bk_f
    biasp[:, 2] = bv_f
    biasp[:, 3] = np.asarray(b_proj, np.float32)
    biasp[:, 4] = np.asarray(b2, np.float32)
    biasp[:, 5:9] = b1_f.reshape(4, C).T
    bvrow = bv_f.reshape(1, C).astype(np.float32)

    tri = np.tril(np.ones((128, 128), np.float32)).T  # [s, q]: 1 iff s <= q
    in_maps = []
    for core in range(8):
        b, half = core // 2, core % 2
        xT = np.ascontiguousarray(x[b].T)
        xTm = np.ascontiguousarray(xT[:, _mycols(half)])
        masks = np.zeros((256, 128), np.float32)
        if half == 0:
            masks[0:128] = tri
            masks[128:256] = 0.0
        else:
            masks[0:128] = 1.0
            masks[128:256] = tri
        in_maps.append({
            "xT": xT, "xTm": xTm,
            "wqkvp": wqkvp, "w1p": w1p, "w2p": w2p,
            "biasp": biasp, "bvrow": bvrow,
            "masks": masks.astype(bf16),
        })
    return in_maps


def _assemble(results, dtype):
    out = np.empty((B, T, C), dtype)
    for core in range(8):
        b, half = core // 2, core % 2
        out[b, _mycols(half), :] = results[core]["outT"].T
    return out


def kernel(**inputs):
    in_maps = _prep_inputs(**inputs)
    if "nc" not in _cache:
        _cache["nc"] = _build()
    res = bass_utils.run_bass_kernel_spmd(_cache["nc"], in_maps,
                                          core_ids=list(range(8)))
    return _assemble(res.results, np.asarray(inputs["x"]).dtype)


# revision 46
# speedup vs baseline: 1.1663x; 1.0229x over previous
"""Transformer block (LN -> 12-head causal attention -> residual -> LN -> MLP
-> residual) for B=4, T=2048, C=768 on 8 trn2 NeuronCores.

Sharding: core = (batch, token-half). Each core handles one batch's K/V in
full and produces the final output for half the tokens (even or odd 128-token
blocks, which balances the causal-attention triangle). No collectives; all
per-core structural differences are carried in input *data* (host-gathered
xTm, causal-boundary mask tiles) so a single SPMD program runs on all 8
cores.

On-chip layout is feature-major ("transposed", [C, T]). LN statistics are
computed with ones-vector matmuls on the tensor engine and the LN1+QKV
pipeline is interleaved per 512-token group so the PE never starves.
Attention processes all 1024 owned query columns per head in one pass
(1024-wide exp tiles halve the scalar-engine instruction count); softmax
row-sums ride a ones column appended to V and are inverted with the fast
Newton-Raphson reciprocal.
"""

import math
import os
import sys

for _p in ("/opt/trn_rl_repo", "/root/.axon_site/_ro/trn_rl_repo"):
    if os.path.isdir(_p) and _p not in sys.path:
        sys.path.append(_p)

import numpy as np
import ml_dtypes

import concourse.bacc as bacc
import concourse.tile as tile
import concourse.mybir as mybir
from concourse import bass_utils
from concourse.alu_op_type import AluOpType
from concourse.tile_rust import add_dep_helper

BF = mybir.dt.bfloat16
FP = mybir.dt.float32
AF = mybir.ActivationFunctionType

B, T, C, H, HD = 4, 2048, 768, 12, 64
EPS = 1e-5
SHIFT = 40.0  # constant softmax shift: exp(s - SHIFT); exact softmax
NP = C // 128  # 6 feature partition-tiles
NT = T // 128  # 16 token blocks
TM = T // 2    # 1024 tokens owned per core
NG = 4         # 512-token groups
bf16 = ml_dtypes.bfloat16

_cache = {}


def _build(debug=False):
    nc = bacc.Bacc("TRN2", target_bir_lowering=False, debug=False)
    d_xT = nc.dram_tensor("xT", [C, T], FP, kind="ExternalInput").ap()
    d_xTm = nc.dram_tensor("xTm", [C, TM], FP, kind="ExternalInput").ap()
    d_wqkvp = nc.dram_tensor("wqkvp", [C, 4 * C], BF, kind="ExternalInput").ap()
    d_w1 = nc.dram_tensor("w1p", [C, 4 * C], BF, kind="ExternalInput").ap()
    d_w2 = nc.dram_tensor("w2p", [C, 4 * C], BF, kind="ExternalInput").ap()
    d_bias = nc.dram_tensor("biasp", [C, 9], FP, kind="ExternalInput").ap()
    d_bvrow = nc.dram_tensor("bvrow", [1, C], FP, kind="ExternalInput").ap()
    d_masks = nc.dram_tensor("masks", [256, 128], BF, kind="ExternalInput").ap()
    d_out = nc.dram_tensor("outT", [C, TM], FP, kind="ExternalOutput").ap()
    if debug is True:
        debug = ["h", "hm", "KT", "QT", "V", "attnT", "xmid"]
    debug = debug or []
    dbg = {}
    if "h" in debug:
        dbg["h"] = nc.dram_tensor("dbg_h", [C, T], BF, kind="ExternalOutput").ap()
    if "hm" in debug:
        dbg["hm"] = nc.dram_tensor("dbg_hm", [C, TM], BF, kind="ExternalOutput").ap()
    if "KT" in debug:
        dbg["KT"] = nc.dram_tensor("dbg_KT", [C, T], BF, kind="ExternalOutput").ap()
    if "QT" in debug:
        dbg["QT"] = nc.dram_tensor("dbg_QT", [C, TM], BF, kind="ExternalOutput").ap()
    if "V" in debug:
        dbg["V"] = nc.dram_tensor("dbg_V", [T, H * 65], BF, kind="ExternalOutput").ap()
    if "attnT" in debug:
        dbg["attnT"] = nc.dram_tensor("dbg_attnT", [C, TM], BF, kind="ExternalOutput").ap()
    if "xmid" in debug:
        dbg["xmid"] = nc.dram_tensor("dbg_xmid", [C, TM], FP, kind="ExternalOutput").ap()

    with tile.TileContext(nc) as tc:
        _body(nc, tc, d_xT, d_xTm, d_wqkvp, d_w1, d_w2, d_bias, d_bvrow,
              d_masks, d_out, dbg)
    nc.compile()
    return nc


def _ln_smalls(nc, small, sum_row, sq_row, ncols, eps_c, a_dst, c_dst):
    """From accumulated sum/sumsq rows [1, ncols], produce bf16 [1, ncols]
    rows a5b (1/std) and c5b (-mu/std)."""
    n = ncols
    mu = small.tile([1, 512], FP, tag="mu", name="mu")[0:1, 0:n]
    nc.scalar.mul(mu, sum_row, 1.0 / C)
    m2 = small.tile([1, 512], FP, tag="m2", name="m2")[0:1, 0:n]
    nc.scalar.mul(m2, sq_row, 1.0 / C)
    var = small.tile([1, 512], FP, tag="va", name="va")[0:1, 0:n]
    nc.vector.tensor_mul(var, mu, mu)
    nc.vector.tensor_sub(var, m2, var)
    std = small.tile([1, 512], FP, tag="sd", name="sd")[0:1, 0:n]
    nc.scalar.activation(std, var, AF.Sqrt, bias=eps_c[0:1, 0:1])
    a5 = small.tile([1, 512], FP, tag="a5", name="a5")[0:1, 0:n]
    nc.vector.reciprocal_approx_accurate(a5, std, var)
    c5 = small.tile([1, 512], FP, tag="c5", name="c5")[0:1, 0:n]
    nc.vector.scalar_tensor_tensor(c5, mu, -1.0, a5,
                                   AluOpType.mult, AluOpType.mult)
    nc.vector.tensor_copy(a_dst[:], a5)
    nc.vector.tensor_copy(c_dst[:], c5)


def _body(nc, tc, d_xT, d_xTm, d_wqkvp, d_w1, d_w2, d_bias, d_bvrow,
          d_masks, d_out, dbg={}):
    from contextlib import ExitStack

    es = ExitStack()
    g_const = es.enter_context(tc.tile_pool(name="const", bufs=1))
    g_xmid = es.enter_context(tc.tile_pool(name="xmid", bufs=1))
    # bf16 residual trunk: halves SBUF so MLP weights can coexist with K/Q/V
    xmid = [g_xmid.tile([128, TM], BF, tag=f"xm{i}", name=f"xm{i}") for i in range(NP)]
    kqv_stack = ExitStack()
    w_es = ExitStack()
    g_kqv = kqv_stack.enter_context(tc.tile_pool(name="kqv", bufs=1))

    # ---- constants ----
    ones_bf = g_const.tile([128, 1], BF, tag="ones_bf", name="ones_bf")
    nc.vector.memset(ones_bf[:], 1.0)
    eps_c = g_const.tile([128, 1], FP, tag="eps_c", name="eps_c")
    nc.vector.memset(eps_c[:], EPS)
    shift_c = g_const.tile([128, 1], FP, tag="shift_c", name="shift_c")
    nc.vector.memset(shift_c[:], -SHIFT)
    bias_sb = [g_const.tile([128, 9], FP, tag=f"bias{f}", name=f"bias{f}") for f in range(NP)]
    for f in range(NP):
        nc.sync.dma_start(bias_sb[f][:], d_bias[f * 128:(f + 1) * 128, :])
    mask_ab = g_const.tile([128, 256], BF, tag="mask_ab", name="mask_ab")
    nc.sync.dma_start(mask_ab[:, 0:128], d_masks[0:128, :])
    nc.sync.dma_start(mask_ab[:, 128:256], d_masks[128:256, :])
    bv_row = g_const.tile([1, C], FP, tag="bv_row", name="bv_row")
    nc.sync.dma_start(bv_row[:], d_bvrow[:])
    bv_rb = g_const.tile([1, C], BF, tag="bv_rb", name="bv_rb")
    nc.vector.tensor_copy(bv_rb[:], bv_row[:])
    bv_bc = g_const.tile([128, C], BF, tag="bv_bc", name="bv_bc")
    nc.gpsimd.partition_broadcast(bv_bc[:], bv_rb[:])

    # ---- persistent activation storage ----
    KT = [g_kqv.tile([128, T], BF, tag=f"KT{i}", name=f"KT{i}") for i in range(NP)]
    QT = [g_kqv.tile([128, TM], BF, tag=f"QT{i}", name=f"QT{i}") for i in range(NP)]
    Vsb = [g_kqv.tile([128, H * 65], BF, tag=f"V{t}", name=f"V{t}") for t in range(NT)]
    wproj = [g_kqv.tile([128, C], BF, tag=f"wp{c}", name=f"wp{c}") for c in range(NP)]

    # ---- pools created up-front in global LIFO order ----
    attnT_es = ExitStack()
    g_attnT = attnT_es.enter_context(tc.tile_pool(name="attnT", bufs=1))
    attnT = [g_attnT.tile([128, TM], BF, tag=f"aT{i}", name=f"aT{i}") for i in range(NP)]

    fusA_es = ExitStack()
    g_wei = fusA_es.enter_context(tc.tile_pool(name="wei", bufs=2))
    g_asc = fusA_es.enter_context(tc.tile_pool(name="ascratch", bufs=1))
    g_rb = fusA_es.enter_context(tc.tile_pool(name="rbpool", bufs=1))
    pa_pool = fusA_es.enter_context(tc.tile_pool(name="aps", bufs=2, space="PSUM"))
    ps0_es = ExitStack()
    ps0_pool = ps0_es.enter_context(tc.tile_pool(name="sps0", bufs=1, space="PSUM"))

    # ---- weights for attention part (pool above att pools so it can close
    # right after the QKV phase) ----
    g_w = w_es.enter_context(tc.tile_pool(name="wqkvp", bufs=1))
    w_sb = [g_w.tile([128, 4 * C], BF, tag=f"w{c}", name=f"w{c}") for c in range(NP)]

    # ================= LN1 + QKV, pipelined per 512-token group =============
    ln_es = ExitStack()
    g_roll = ln_es.enter_context(tc.tile_pool(name="lnroll", bufs=2))
    g_bc = ln_es.enter_context(tc.tile_pool(name="lnbc", bufs=2))
    g_small = ln_es.enter_context(tc.tile_pool(name="lnsmall", bufs=1))
    sps = ln_es.enter_context(tc.tile_pool(name="statps", bufs=1, space="PSUM"))
    gps = ln_es.enter_context(tc.tile_pool(name="gemmps", bufs=3, space="PSUM"))

    def emit_ln_group(g):
        """LN1 stats + h/hm for token group g (cols g*512..(g+1)*512 of T,
        owned cols g*256..(g+1)*256 of TM). Returns per-group h/hm tiles."""
        csl = slice(g * 512, (g + 1) * 512)
        msl = slice(g * 256, (g + 1) * 256)
        allst = sps.tile([128, 512], FP, tag="stf", name="allst")
        stats = allst[0:33, :]
        xbs, xbms = [], []
        for ci in range(NP):
            xt = g_roll.tile([128, 512], FP, tag="xr", name="xr")
            nc.sync.dma_start(xt[:], d_xT[ci * 128:(ci + 1) * 128, csl])
            xb = g_roll.tile([128, 512], BF, tag="xb", name="xb", bufs=6)
            nc.vector.tensor_copy(xb[:], xt[:])
            sq = g_roll.tile([128, 512], BF, tag="sq", name="sq")
            nc.vector.tensor_mul(sq[:], xb[:], xb[:])
            nc.tensor.matmul(stats[0:1, :], ones_bf[:], xb[:],
                             start=(ci == 0), stop=(ci == NP - 1),
                             skip_group_check=True)
            nc.tensor.matmul(stats[32:33, :], ones_bf[:], sq[:],
                             start=(ci == 0), stop=(ci == NP - 1),
                             skip_group_check=True)
            xbs.append(xb)
            xtm = g_roll.tile([128, 256], FP, tag="xrm", name="xrm")
            nc.sync.dma_start(xtm[:], d_xTm[ci * 128:(ci + 1) * 128, msl])
            xbm = g_roll.tile([128, 256], BF, tag="xbm", name="xbm", bufs=6)
            nc.vector.tensor_copy(xbm[:], xtm[:])
            sqm = g_roll.tile([128, 256], BF, tag="sqm", name="sqm")
            nc.vector.tensor_mul(sqm[:], xbm[:], xbm[:])
            nc.tensor.matmul(allst[64:65, 0:256], ones_bf[:], xbm[:],
                             start=(ci == 0), stop=(ci == NP - 1),
                             skip_group_check=True)
            nc.tensor.matmul(allst[64:65, 256:512], ones_bf[:], sqm[:],
                             start=(ci == 0), stop=(ci == NP - 1),
                             skip_group_check=True)
            xbms.append(xbm)
        a_row = g_small.tile([1, 512], BF, tag="a_row", name="a_row")
        c_row = g_small.tile([1, 512], BF, tag="c_row", name="c_row")
        _ln_smalls(nc, g_small, stats[0:1, :], stats[32:33, :], 512, eps_c, a_row, c_row)
        a_bc = g_bc.tile([128, 512], BF, tag="a_bc", name="a_bc")
        c_bc = g_bc.tile([128, 512], BF, tag="c_bc", name="c_bc")
        nc.gpsimd.partition_broadcast(a_bc[:], a_row[:])
        nc.gpsimd.partition_broadcast(c_bc[:], c_row[:])
        am_row = g_small.tile([1, 512], BF, tag="a_row", name="am_row")[0:1, 0:256]
        cm_row = g_small.tile([1, 512], BF, tag="c_row", name="cm_row")[0:1, 0:256]
        _ln_smalls(nc, g_small, allst[64:65, 0:256], allst[64:65, 256:512], 256, eps_c, am_row, cm_row)
        am_bc = g_bc.tile([128, 256], BF, tag="am_bc", name="am_bc")
        cm_bc = g_bc.tile([128, 256], BF, tag="cm_bc", name="cm_bc")
        nc.gpsimd.partition_broadcast(am_bc[:], am_row[:])
        nc.gpsimd.partition_broadcast(cm_bc[:], cm_row[:])
        hg, hmg = [], []
        for ci in range(NP):
            tmp = g_roll.tile([128, 512], BF, tag="h_tmp", name="h_tmp")
            nc.vector.tensor_mul(tmp[:], xbs[ci][:], a_bc[:])
            hgc = g_roll.tile([128, 512], BF, tag=f"h{ci}", name="hgc")
            nc.vector.tensor_add(hgc[:], tmp[:], c_bc[:])
            hg.append(hgc)
            tmpm = g_roll.tile([128, 256], BF, tag="hm_tmp", name="hm_tmp")
            nc.vector.tensor_mul(tmpm[:], xbms[ci][:], am_bc[:])
            hmgc = g_roll.tile([128, 256], BF, tag=f"hm{ci}", name="hmgc")
            nc.vector.tensor_add(hmgc[:], tmpm[:], cm_bc[:])
            hmg.append(hmgc)
        if "h" in dbg:
            for ci in range(NP):
                nc.sync.dma_start(dbg["h"][ci * 128:(ci + 1) * 128, csl], hg[ci][:])
        if "hm" in dbg:
            for ci in range(NP):
                nc.sync.dma_start(dbg["hm"][ci * 128:(ci + 1) * 128, msl], hmg[ci][:])
        return hg, hmg

    def emit_qkv_group(g, hg, hmg, part="kqv"):
        """K/Q/V GEMMs for token group g."""
        csl = slice(g * 512, (g + 1) * 512)
        msl = slice(g * 256, (g + 1) * 256)
        if part == "v":
            emit_v_group(g, hg)
            return
        for f in range(NP):
            ps = gps.tile([128, 512], FP, tag="ps", name="ps")
            for c in range(NP):
                nc.tensor.matmul(ps[:], w_sb[c][:, C + f * 128:C + (f + 1) * 128],
                                 hg[c][:], start=(c == 0), stop=(c == NP - 1))
            nc.scalar.activation(KT[f][:, csl], ps[:], AF.Identity,
                                 bias=bias_sb[f][:, 1:2])
        for f in range(NP):
            ps = gps.tile([128, 512], FP, tag="ps", name="ps")
            for c in range(NP):
                nc.tensor.matmul(ps[:, 0:256], w_sb[c][:, f * 128:(f + 1) * 128],
                                 hmg[c][:], start=(c == 0), stop=(c == NP - 1))
            nc.scalar.activation(QT[f][:, msl], ps[:, 0:256], AF.Identity,
                                 bias=bias_sb[f][:, 0:1])
        if part == "kq":
            return
        emit_v_group(g, hg)

    def emit_v_group(g, hg):
        for t in range(4):
            v3 = Vsb[4 * g + t][:].rearrange("p (h d) -> p h d", d=65)
            nc.vector.memset(v3[:, :, 64:65], 1.0)
            for fs in range(2):
                n = 512 if fs == 0 else 256
                nh = n // 64
                ps = gps.tile([128, 512], FP, tag="ps", name="ps")
                for c in range(NP):
                    nc.tensor.matmul(ps[:, 0:n], hg[c][:, t * 128:(t + 1) * 128],
                                     w_sb[c][:, 2 * C + fs * 512:2 * C + fs * 512 + n],
                                     start=(c == 0), stop=(c == NP - 1))
                nc.vector.scalar_tensor_tensor(
                    v3[:, fs * 8:fs * 8 + nh, 0:64],
                    ps[:, 0:n].rearrange("p (h d) -> p h d", d=64),
                    0.0,
                    bv_bc[:, fs * 512:fs * 512 + n].rearrange("p (h d) -> p h d", d=64),
                    AluOpType.add, AluOpType.add)

    def att_head(hh, g, pspool):
        """Causal attention for head hh over query col group g (512 cols).
        Key blocks processed in pairs sharing one [128,1024] PSUM tile so a
        single exp instruction covers both (3D access pattern)."""
        ht, hp = hh // 2, (hh % 2) * 64
        npairs = 4 + 4 * g
        pa = pa_pool.tile([65, 512], FP, tag="pa", name="pa")
        for u in range(npairs):
            sa, sb = 2 * u, 2 * u + 1
            c0 = max(0, u - 4 * g) * 128
            ps = pspool.tile([128, 1024], FP, tag="ps", name="ps")
            qs = QT[ht][hp:hp + 64, g * 512 + c0:(g + 1) * 512]
            nc.tensor.matmul(ps[:, c0:512],
                             KT[ht][hp:hp + 64, sa * 128:(sa + 1) * 128], qs,
                             start=True, stop=True)
            nc.tensor.matmul(ps[:, 512 + c0:1024],
                             KT[ht][hp:hp + 64, sb * 128:(sb + 1) * 128], qs,
                             start=True, stop=True)
            wei = g_wei.tile([128, 1024], BF, tag="wei", name="wei")
            ps3 = ps[:].rearrange("p (two q) -> p two q", two=2)
            we3 = wei[:].rearrange("p (two q) -> p two q", two=2)
            nc.scalar.activation(we3[:, :, c0:512], ps3[:, :, c0:512], AF.Exp,
                                 bias=shift_c[:])
            jd = u - 4 * g
            if 0 <= jd < 4:
                nc.vector.tensor_mul(
                    we3[:, :, jd * 128:(jd + 1) * 128],
                    we3[:, :, jd * 128:(jd + 1) * 128],
                    mask_ab[:].rearrange("p (two q) -> p two q", two=2))
            nc.tensor.matmul(pa[:, c0:512], Vsb[sa][:, hh * 65:(hh + 1) * 65],
                             wei[:, c0:512], start=(u == 0), stop=False,
                             skip_group_check=True)
            nc.tensor.matmul(pa[:, c0:512], Vsb[sb][:, hh * 65:(hh + 1) * 65],
                             wei[:, 512 + c0:1024], start=False,
                             stop=(u == npairs - 1), skip_group_check=True)
        sumrow = g_asc.tile([1, 512], FP, tag="sumrow", name="sumrow")
        nc.vector.tensor_copy(sumrow[:], pa[64:65, :])
        recip = g_asc.tile([1, 512], FP, tag="recip", name="recip")
        scr = g_asc.tile([1, 512], FP, tag="scr", name="scr")
        nc.vector.reciprocal_approx_accurate(recip[:], sumrow[:], scr[:])
        rb = g_rb.tile([64, 512], FP, tag="rb", name="rb")
        nc.gpsimd.partition_broadcast(rb[:], recip[:])
        nc.vector.tensor_mul(attnT[ht][hp:hp + 64, g * 512:(g + 1) * 512],
                             pa[0:64, :], rb[:])

    # phase 1 pipeline with attention group 0 overlapped onto its tail
    cur = {0: emit_ln_group(0)}
    cur[1] = emit_ln_group(1)
    for c in range(NP):
        nc.sync.dma_start(w_sb[c][:], d_wqkvp[c * 128:(c + 1) * 128, :])
    emit_qkv_group(0, *cur[0])
    cur[2] = emit_ln_group(2)
    emit_qkv_group(1, *cur[1])
    for hh in range(H):
        att_head(hh, 0, ps0_pool)
        if hh == 0:
            cur[3] = emit_ln_group(3)
        elif hh == 2:
            emit_qkv_group(2, *cur[2], part="kq")
        elif hh == 4:
            emit_qkv_group(2, *cur[2], part="v")
        elif hh == 6:
            emit_qkv_group(3, *cur[3], part="kq")
        elif hh == 8:
            emit_qkv_group(3, *cur[3], part="v")
            for c in range(NP):
                nc.vector.tensor_copy(wproj[c][:], w_sb[c][:, 3 * C:4 * C])

    for c in range(NP):
        if "KT" in dbg:
            nc.sync.dma_start(dbg["KT"][c * 128:(c + 1) * 128, :], KT[c][:])
        if "QT" in dbg:
            nc.sync.dma_start(dbg["QT"][c * 128:(c + 1) * 128, :], QT[c][:])
    if "V" in dbg:
        for t in range(NT):
            nc.sync.dma_start(dbg["V"][t * 128:(t + 1) * 128, :], Vsb[t][:])
    ln_es.close()
    w_es.close()
    ps0_es.close()

    # ========== fused attention(group 1) + proj + LN2 + MLP ==========
    fus_es = ExitStack()
    g_w1 = fus_es.enter_context(tc.tile_pool(name="w1p", bufs=1))
    g_h2 = fus_es.enter_context(tc.tile_pool(name="h2p", bufs=1))
    g_r = fus_es.enter_context(tc.tile_pool(name="rp", bufs=1))
    g_roll2 = fus_es.enter_context(tc.tile_pool(name="mlproll", bufs=2))
    g_bc2 = fus_es.enter_context(tc.tile_pool(name="mlpbc", bufs=1))
    g_small2 = fus_es.enter_context(tc.tile_pool(name="mlpsmall", bufs=1))
    ps1_pool = fus_es.enter_context(tc.tile_pool(name="sps1", bufs=2, space="PSUM"))
    gps3 = fus_es.enter_context(tc.tile_pool(name="mps", bufs=2, space="PSUM"))

    w1_sb = [g_w1.tile([128, 4 * C], BF, tag=f"w1_{c}", name=f"w1_{c}") for c in range(NP)]
    for c in range(NP):
        nc.sync.dma_start(w1_sb[c][:], d_w1[c * 128:(c + 1) * 128, :])

    def emit_proj(g):
        gsl = slice(g * 512, (g + 1) * 512)
        for f in range(NP):
            ps = gps3.tile([128, 512], FP, tag="ps", name="ps")
            for c in range(NP):
                nc.tensor.matmul(ps[:], wproj[c][:, f * 128:(f + 1) * 128],
                                 attnT[c][:, gsl],
                                 start=(c == 0), stop=(c == NP - 1))
            xr = g_roll2.tile([128, 512], FP, tag="xr", name="xr")
            nc.sync.dma_start(xr[:], d_xTm[f * 128:(f + 1) * 128, gsl])
            nc.vector.scalar_tensor_tensor(
                xmid[f][:, gsl], ps[:], bias_sb[f][:, 3:4],
                xr[:], AluOpType.add, AluOpType.add)

    h2 = {}

    def emit_ln2(g):
        gsl = slice(g * 512, (g + 1) * 512)
        stats = pa_pool.tile([65, 512], FP, tag="pa", name="st2")
        for ci in range(NP):
            sq = g_roll2.tile([128, 512], BF, tag="sq2", name="sq2")
            nc.vector.tensor_mul(sq[:], xmid[ci][:, gsl], xmid[ci][:, gsl])
            nc.tensor.matmul(stats[0:1, :], ones_bf[:], xmid[ci][:, gsl],
                             start=(ci == 0), stop=(ci == NP - 1),
                             skip_group_check=True)
            nc.tensor.matmul(stats[32:33, :], ones_bf[:], sq[:],
                             start=(ci == 0), stop=(ci == NP - 1),
                             skip_group_check=True)
        a_row2 = g_small2.tile([1, 512], BF, tag="a_row2", name="a_row2")
        c_row2 = g_small2.tile([1, 512], BF, tag="c_row2", name="c_row2")
        _ln_smalls(nc, g_small2, stats[0:1, :], stats[32:33, :], 512, eps_c, a_row2, c_row2)
        a_bc2 = g_bc2.tile([128, 512], BF, tag="a2bc", name="a2bc")
        c_bc2 = g_bc2.tile([128, 512], BF, tag="c2bc", name="c2bc")
        nc.gpsimd.partition_broadcast(a_bc2[:], a_row2[:])
        nc.gpsimd.partition_broadcast(c_bc2[:], c_row2[:])
        hts = []
        for ci in range(NP):
            tmp = g_roll2.tile([128, 512], BF, tag="h2tmp", name="h2tmp")
            nc.vector.tensor_mul(tmp[:], xmid[ci][:, gsl], a_bc2[:])
            hh2 = g_h2.tile([128, 512], BF, tag=f"h2_{ci}", name=f"h2_{ci}")
            nc.vector.tensor_add(hh2[:], tmp[:], c_bc2[:])
            hts.append(hh2)
        h2[g] = hts

    r_tiles = {}

    def emit_fc1(g, m0, m1):
        for m in range(m0, m1):
            ps = gps3.tile([128, 512], FP, tag="ps", name="ps")
            for c in range(NP):
                nc.tensor.matmul(ps[:], w1_sb[c][:, m * 128:(m + 1) * 128],
                                 h2[g][c][:],
                                 start=(c == 0), stop=(c == NP - 1))
            r = g_r.tile([128, 512], BF, tag=f"r{m}", name=f"r{m}")
            nc.vector.tensor_scalar(r[:], ps[:],
                                    bias_sb[m % 6][:, 5 + m // 6:6 + m // 6],
                                    0.0, AluOpType.add, AluOpType.max)
            r_tiles[g, m] = r

    w2k = [None] * 6
    w2q = [None] * 6

    def emit_w2_chunk(i):
        """Load w2 columns into the KT[i]/QT[i] buffers (pool-tag reuse) as
        soon as heads 2i/2i+1 have finished reading them."""
        w2k[i] = g_kqv.tile([128, 2048], BF, tag=f"KT{i}", name=f"w2k{i}")
        g0, g1 = i * 2048, (i + 1) * 2048
        st = g0
        while st < g1:
            c = st // 3072
            en = min(g1, (c + 1) * 3072)
            nc.sync.dma_start(w2k[i][:, st - g0:en - g0],
                              d_w2[c * 128:(c + 1) * 128,
                                   st - c * 3072:en - c * 3072])
            st = en
        w2q[i] = g_kqv.tile([128, 1024], BF, tag=f"QT{i}", name=f"w2q{i}")
        q0 = 12288 + 1024 * i
        c = q0 // 3072
        nc.sync.dma_start(w2q[i][:], d_w2[c * 128:(c + 1) * 128,
                                          q0 - c * 3072:q0 - c * 3072 + 1024])

    def w2_slice(m, f):
        g0 = (m // 4) * 3072 + (m % 4) * 768 + f * 128
        if g0 < 12288:
            return w2k[g0 // 2048][:, g0 % 2048:g0 % 2048 + 128]
        rr = g0 - 12288
        return w2q[rr // 1024][:, rr % 1024:rr % 1024 + 128]

    def emit_fc2(g):
        gsl = slice(g * 512, (g + 1) * 512)
        for f in range(NP):
            ps = gps3.tile([128, 512], FP, tag="ps", name="ps")
            for m in range(24):
                nc.tensor.matmul(ps[:], w2_slice(m, f), r_tiles[g, m][:],
                                 start=(m == 0), stop=(m == 23))
            ot = g_roll2.tile([128, 512], FP, tag="ot", name="ot")
            nc.vector.scalar_tensor_tensor(ot[:], ps[:], bias_sb[f][:, 4:5],
                                           xmid[f][:, gsl],
                                           AluOpType.add, AluOpType.add)
            nc.sync.dma_start(d_out[f * 128:(f + 1) * 128, gsl], ot[:])

    # attention group 1 interleaved with proj/LN2/fc1 of group 0 (PE keeps
    # dense work and full clock while the exp chain runs) and with w2 loads
    work = {2: lambda: emit_proj(0), 3: lambda: emit_ln2(0),
            4: lambda: emit_fc1(0, 0, 6), 6: lambda: emit_fc1(0, 6, 12),
            8: lambda: emit_fc1(0, 12, 18), 10: lambda: emit_fc1(0, 18, 24)}
    for hh in range(H):
        att_head(hh, 1, ps1_pool)
        if hh % 2 == 1:
            emit_w2_chunk(hh // 2)
        if hh in work:
            work[hh]()
    if "attnT" in dbg:
        for c in range(NP):
            nc.sync.dma_start(dbg["attnT"][c * 128:(c + 1) * 128, :], attnT[c][:])

    emit_proj(1)
    emit_ln2(1)
    emit_fc2(0)
    emit_fc1(1, 0, 24)
    emit_fc2(1)
    if "xmid" in dbg:
        for c in range(NP):
            nc.sync.dma_start(dbg["xmid"][c * 128:(c + 1) * 128, :], xmid[c][:])
    fus_es.close()
    fusA_es.close()
    attnT_es.close()
    kqv_stack.close()
    es.close()


# ---------------------------------------------------------------------------
# host side
# ---------------------------------------------------------------------------

def _mycols(half):
    blocks = np.arange(8) * 2 + half
    return (blocks[:, None] * 128 + np.arange(128)[None, :]).reshape(-1)


def _prep_inputs(x, wq, bq, wk, bk, wv, bv, w_proj, b_proj, w1, b1, w2, b2,
                 g1, beta1, g2, beta2):
    x = np.asarray(x, np.float32)
    wq_f = np.ascontiguousarray(np.transpose(np.asarray(wq, np.float32), (1, 0, 2)).reshape(C, C))
    wk_f = np.ascontiguousarray(np.transpose(np.asarray(wk, np.float32), (1, 0, 2)).reshape(C, C))
    wv_f = np.ascontiguousarray(np.transpose(np.asarray(wv, np.float32), (1, 0, 2)).reshape(C, C))
    g1 = np.asarray(g1, np.float32); beta1 = np.asarray(beta1, np.float32)
    g2 = np.asarray(g2, np.float32); beta2 = np.asarray(beta2, np.float32)
    w1 = np.asarray(w1, np.float32); w2 = np.asarray(w2, np.float32)
    w_proj = np.asarray(w_proj, np.float32)

    wq_g = g1[:, None] * wq_f
    wk_g = g1[:, None] * wk_f
    wv_g = g1[:, None] * wv_f
    bq_f = beta1 @ wq_f + np.asarray(bq, np.float32).reshape(-1)
    bk_f = beta1 @ wk_f + np.asarray(bk, np.float32).reshape(-1)
    bv_f = beta1 @ wv_f + np.asarray(bv, np.float32).reshape(-1)
    w1_g = g2[:, None] * w1
    b1_f = beta2 @ w1 + np.asarray(b1, np.float32)

    wqkvp = np.concatenate([wq_g, wk_g, wv_g, w_proj], axis=1).astype(bf16)
    w1p = w1_g.astype(bf16)
    w2p = np.ascontiguousarray(
        w2.reshape(6, 4, 128, C).transpose(0, 2, 1, 3).reshape(C, 4 * C)).astype(bf16)

    biasp = np.zeros((C, 9), np.float32)
    biasp[:, 0] = bq_f
    biasp[:, 1] = bk_f
    biasp[:, 2] = bv_f
    biasp[:, 3] = np.asarray(b_proj, np.float32)
    biasp[:, 4] = np.asarray(b2, np.float32)
    biasp[:, 5:9] = b1_f.reshape(4, C).T
    bvrow = bv_f.reshape(1, C).astype(np.float32)

    tri = np.tril(np.ones((128, 128), np.float32)).T  # [s, q]: 1 iff s <= q
    in_maps = []
    for core in range(8):
        b, half = core // 2, core % 2
        xT = np.ascontiguousarray(x[b].T)
        xTm = np.ascontiguousarray(xT[:, _mycols(half)])
        masks = np.zeros((256, 128), np.float32)
        if half == 0:
            masks[0:128] = tri
            masks[128:256] = 0.0
        else:
            masks[0:128] = 1.0
            masks[128:256] = tri
        in_maps.append({
            "xT": xT, "xTm": xTm,
            "wqkvp": wqkvp, "w1p": w1p, "w2p": w2p,
            "biasp": biasp, "bvrow": bvrow,
            "masks": masks.astype(bf16),
        })
    return in_maps


def _assemble(results, dtype):
    out = np.empty((B, T, C), dtype)
    for core in range(8):
        b, half = core // 2, core % 2
        out[b, _mycols(half), :] = results[core]["outT"].T
    return out


def kernel(**inputs):
    in_maps = _prep_inputs(**inputs)
    if "nc" not in _cache:
        _cache["nc"] = _build()
    res = bass_utils.run_bass_kernel_spmd(_cache["nc"], in_maps,
                                          core_ids=list(range(8)))
    return _assemble(res.results, np.asarray(inputs["x"]).dtype)


# revision 48
# speedup vs baseline: 1.1688x; 1.0021x over previous
"""Transformer block (LN -> 12-head causal attention -> residual -> LN -> MLP
-> residual) for B=4, T=2048, C=768 on 8 trn2 NeuronCores.

Sharding: core = (batch, token-half). Each core handles one batch's K/V in
full and produces the final output for half the tokens (even or odd 128-token
blocks, which balances the causal-attention triangle). No collectives; all
per-core structural differences are carried in input *data* (host-gathered
xTm, causal-boundary mask tiles) so a single SPMD program runs on all 8
cores.

On-chip layout is feature-major ("transposed", [C, T]). LN statistics are
computed with ones-vector matmuls on the tensor engine and the LN1+QKV
pipeline is interleaved per 512-token group so the PE never starves.
Attention processes all 1024 owned query columns per head in one pass
(1024-wide exp tiles halve the scalar-engine instruction count); softmax
row-sums ride a ones column appended to V and are inverted with the fast
Newton-Raphson reciprocal.
"""

import math
import os
import sys

for _p in ("/opt/trn_rl_repo", "/root/.axon_site/_ro/trn_rl_repo"):
    if os.path.isdir(_p) and _p not in sys.path:
        sys.path.append(_p)

import numpy as np
import ml_dtypes

import concourse.bacc as bacc
import concourse.tile as tile
import concourse.mybir as mybir
from concourse import bass_utils
from concourse.alu_op_type import AluOpType
from concourse.tile_rust import add_dep_helper

BF = mybir.dt.bfloat16
FP = mybir.dt.float32
AF = mybir.ActivationFunctionType

B, T, C, H, HD = 4, 2048, 768, 12, 64
EPS = 1e-5
SHIFT = 40.0  # constant softmax shift: exp(s - SHIFT); exact softmax
NP = C // 128  # 6 feature partition-tiles
NT = T // 128  # 16 token blocks
TM = T // 2    # 1024 tokens owned per core
NG = 4         # 512-token groups
bf16 = ml_dtypes.bfloat16

_cache = {}


def _build(debug=False):
    nc = bacc.Bacc("TRN2", target_bir_lowering=False, debug=False)
    d_xT = nc.dram_tensor("xT", [C, T], FP, kind="ExternalInput").ap()
    d_xTm = nc.dram_tensor("xTm", [C, TM], FP, kind="ExternalInput").ap()
    d_wqkvp = nc.dram_tensor("wqkvp", [C, 4 * C], BF, kind="ExternalInput").ap()
    d_w1 = nc.dram_tensor("w1p", [C, 4 * C], BF, kind="ExternalInput").ap()
    d_w2 = nc.dram_tensor("w2p", [C, 4 * C], BF, kind="ExternalInput").ap()
    d_bias = nc.dram_tensor("biasp", [C, 9], FP, kind="ExternalInput").ap()
    d_bvrow = nc.dram_tensor("bvrow", [1, C], FP, kind="ExternalInput").ap()
    d_masks = nc.dram_tensor("masks", [256, 128], BF, kind="ExternalInput").ap()
    d_out = nc.dram_tensor("outT", [C, TM], FP, kind="ExternalOutput").ap()
    if debug is True:
        debug = ["h", "hm", "KT", "QT", "V", "attnT", "xmid"]
    debug = debug or []
    dbg = {}
    if "h" in debug:
        dbg["h"] = nc.dram_tensor("dbg_h", [C, T], BF, kind="ExternalOutput").ap()
    if "hm" in debug:
        dbg["hm"] = nc.dram_tensor("dbg_hm", [C, TM], BF, kind="ExternalOutput").ap()
    if "KT" in debug:
        dbg["KT"] = nc.dram_tensor("dbg_KT", [C, T], BF, kind="ExternalOutput").ap()
    if "QT" in debug:
        dbg["QT"] = nc.dram_tensor("dbg_QT", [C, TM], BF, kind="ExternalOutput").ap()
    if "V" in debug:
        dbg["V"] = nc.dram_tensor("dbg_V", [T, H * 65], BF, kind="ExternalOutput").ap()
    if "attnT" in debug:
        dbg["attnT"] = nc.dram_tensor("dbg_attnT", [C, TM], BF, kind="ExternalOutput").ap()
    if "xmid" in debug:
        dbg["xmid"] = nc.dram_tensor("dbg_xmid", [C, TM], FP, kind="ExternalOutput").ap()

    with tile.TileContext(nc) as tc:
        _body(nc, tc, d_xT, d_xTm, d_wqkvp, d_w1, d_w2, d_bias, d_bvrow,
              d_masks, d_out, dbg)
    nc.compile()
    return nc


def _ln_smalls(nc, small, sum_row, sq_row, ncols, eps_c, a_dst, c_dst):
    """From accumulated sum/sumsq rows [1, ncols], produce bf16 [1, ncols]
    rows a5b (1/std) and c5b (-mu/std)."""
    n = ncols
    mu = small.tile([1, 512], FP, tag="mu", name="mu")[0:1, 0:n]
    nc.scalar.mul(mu, sum_row, 1.0 / C)
    m2 = small.tile([1, 512], FP, tag="m2", name="m2")[0:1, 0:n]
    nc.scalar.mul(m2, sq_row, 1.0 / C)
    var = small.tile([1, 512], FP, tag="va", name="va")[0:1, 0:n]
    nc.vector.tensor_mul(var, mu, mu)
    nc.vector.tensor_sub(var, m2, var)
    std = small.tile([1, 512], FP, tag="sd", name="sd")[0:1, 0:n]
    nc.scalar.activation(std, var, AF.Sqrt, bias=eps_c[0:1, 0:1])
    a5 = small.tile([1, 512], FP, tag="a5", name="a5")[0:1, 0:n]
    nc.vector.reciprocal_approx_accurate(a5, std, var)
    c5 = small.tile([1, 512], FP, tag="c5", name="c5")[0:1, 0:n]
    nc.vector.scalar_tensor_tensor(c5, mu, -1.0, a5,
                                   AluOpType.mult, AluOpType.mult)
    nc.vector.tensor_copy(a_dst[:], a5)
    nc.vector.tensor_copy(c_dst[:], c5)


def _body(nc, tc, d_xT, d_xTm, d_wqkvp, d_w1, d_w2, d_bias, d_bvrow,
          d_masks, d_out, dbg={}):
    from contextlib import ExitStack

    es = ExitStack()
    g_const = es.enter_context(tc.tile_pool(name="const", bufs=1))
    g_xmid = es.enter_context(tc.tile_pool(name="xmid", bufs=1))
    # bf16 residual trunk: halves SBUF so MLP weights can coexist with K/Q/V
    xmid = [g_xmid.tile([128, TM], BF, tag=f"xm{i}", name=f"xm{i}") for i in range(NP)]
    kqv_stack = ExitStack()
    w_es = ExitStack()
    g_kqv = kqv_stack.enter_context(tc.tile_pool(name="kqv", bufs=1))

    # ---- constants ----
    ones_bf = g_const.tile([128, 1], BF, tag="ones_bf", name="ones_bf")
    nc.vector.memset(ones_bf[:], 1.0)
    eps_c = g_const.tile([128, 1], FP, tag="eps_c", name="eps_c")
    nc.vector.memset(eps_c[:], EPS)
    shift_c = g_const.tile([128, 1], FP, tag="shift_c", name="shift_c")
    nc.vector.memset(shift_c[:], -SHIFT)
    bias_sb = [g_const.tile([128, 9], FP, tag=f"bias{f}", name=f"bias{f}") for f in range(NP)]
    for f in range(NP):
        nc.sync.dma_start(bias_sb[f][:], d_bias[f * 128:(f + 1) * 128, :])
    mask_ab = g_const.tile([128, 256], BF, tag="mask_ab", name="mask_ab")
    nc.sync.dma_start(mask_ab[:, 0:128], d_masks[0:128, :])
    nc.sync.dma_start(mask_ab[:, 128:256], d_masks[128:256, :])
    bv_row = g_const.tile([1, C], FP, tag="bv_row", name="bv_row")
    nc.sync.dma_start(bv_row[:], d_bvrow[:])
    bv_rb = g_const.tile([1, C], BF, tag="bv_rb", name="bv_rb")
    nc.vector.tensor_copy(bv_rb[:], bv_row[:])
    bv_bc = g_const.tile([128, C], BF, tag="bv_bc", name="bv_bc")
    nc.gpsimd.partition_broadcast(bv_bc[:], bv_rb[:])

    # ---- persistent activation storage ----
    KT = [g_kqv.tile([128, T], BF, tag=f"KT{i}", name=f"KT{i}") for i in range(NP)]
    QT = [g_kqv.tile([128, TM], BF, tag=f"QT{i}", name=f"QT{i}") for i in range(NP)]
    Vsb = [g_kqv.tile([128, H * 65], BF, tag=f"V{t}", name=f"V{t}") for t in range(NT)]
    wproj = [g_kqv.tile([128, C], BF, tag=f"wp{c}", name=f"wp{c}") for c in range(NP)]

    # ---- pools created up-front in global LIFO order ----
    attnT_es = ExitStack()
    g_attnT = attnT_es.enter_context(tc.tile_pool(name="attnT", bufs=1))
    attnT = [g_attnT.tile([128, TM], BF, tag=f"aT{i}", name=f"aT{i}") for i in range(NP)]

    fusA_es = ExitStack()
    g_wei = fusA_es.enter_context(tc.tile_pool(name="wei", bufs=2))
    g_asc = fusA_es.enter_context(tc.tile_pool(name="ascratch", bufs=1))
    g_rb = fusA_es.enter_context(tc.tile_pool(name="rbpool", bufs=1))
    pa_pool = fusA_es.enter_context(tc.tile_pool(name="aps", bufs=2, space="PSUM"))
    ps0_es = ExitStack()
    ps0_pool = ps0_es.enter_context(tc.tile_pool(name="sps0", bufs=1, space="PSUM"))

    # ---- weights for attention part (pool above att pools so it can close
    # right after the QKV phase) ----
    g_w = w_es.enter_context(tc.tile_pool(name="wqkvp", bufs=1))
    w_sb = [g_w.tile([128, 4 * C], BF, tag=f"w{c}", name=f"w{c}") for c in range(NP)]

    # ================= LN1 + QKV, pipelined per 512-token group =============
    ln_es = ExitStack()
    g_roll = ln_es.enter_context(tc.tile_pool(name="lnroll", bufs=2))
    g_bc = ln_es.enter_context(tc.tile_pool(name="lnbc", bufs=2))
    g_small = ln_es.enter_context(tc.tile_pool(name="lnsmall", bufs=1))
    sps = ln_es.enter_context(tc.tile_pool(name="statps", bufs=1, space="PSUM"))
    gps = ln_es.enter_context(tc.tile_pool(name="gemmps", bufs=3, space="PSUM"))

    def emit_ln_group(g):
        """LN1 stats + h/hm for token group g (cols g*512..(g+1)*512 of T,
        owned cols g*256..(g+1)*256 of TM). Returns per-group h/hm tiles."""
        csl = slice(g * 512, (g + 1) * 512)
        msl = slice(g * 256, (g + 1) * 256)
        allst = sps.tile([128, 512], FP, tag="stf", name="allst")
        stats = allst[0:33, :]
        xbs, xbms = [], []
        for ci in range(NP):
            xt = g_roll.tile([128, 512], FP, tag="xr", name="xr")
            nc.sync.dma_start(xt[:], d_xT[ci * 128:(ci + 1) * 128, csl])
            xb = g_roll.tile([128, 512], BF, tag="xb", name="xb", bufs=6)
            nc.vector.tensor_copy(xb[:], xt[:])
            sq = g_roll.tile([128, 512], BF, tag="sq", name="sq")
            nc.vector.tensor_mul(sq[:], xb[:], xb[:])
            nc.tensor.matmul(stats[0:1, :], ones_bf[:], xb[:],
                             start=(ci == 0), stop=(ci == NP - 1),
                             skip_group_check=True)
            nc.tensor.matmul(stats[32:33, :], ones_bf[:], sq[:],
                             start=(ci == 0), stop=(ci == NP - 1),
                             skip_group_check=True)
            xbs.append(xb)
            xtm = g_roll.tile([128, 256], FP, tag="xrm", name="xrm")
            nc.sync.dma_start(xtm[:], d_xTm[ci * 128:(ci + 1) * 128, msl])
            xbm = g_roll.tile([128, 256], BF, tag="xbm", name="xbm", bufs=6)
            nc.vector.tensor_copy(xbm[:], xtm[:])
            sqm = g_roll.tile([128, 256], BF, tag="sqm", name="sqm")
            nc.vector.tensor_mul(sqm[:], xbm[:], xbm[:])
            nc.tensor.matmul(allst[64:65, 0:256], ones_bf[:], xbm[:],
                             start=(ci == 0), stop=(ci == NP - 1),
                             skip_group_check=True)
            nc.tensor.matmul(allst[64:65, 256:512], ones_bf[:], sqm[:],
                             start=(ci == 0), stop=(ci == NP - 1),
                             skip_group_check=True)
            xbms.append(xbm)
        a_row = g_small.tile([1, 512], BF, tag="a_row", name="a_row")
        c_row = g_small.tile([1, 512], BF, tag="c_row", name="c_row")
        _ln_smalls(nc, g_small, stats[0:1, :], stats[32:33, :], 512, eps_c, a_row, c_row)
        a_bc = g_bc.tile([128, 512], BF, tag="a_bc", name="a_bc")
        c_bc = g_bc.tile([128, 512], BF, tag="c_bc", name="c_bc")
        nc.gpsimd.partition_broadcast(a_bc[:], a_row[:])
        nc.gpsimd.partition_broadcast(c_bc[:], c_row[:])
        am_row = g_small.tile([1, 512], BF, tag="a_row", name="am_row")[0:1, 0:256]
        cm_row = g_small.tile([1, 512], BF, tag="c_row", name="cm_row")[0:1, 0:256]
        _ln_smalls(nc, g_small, allst[64:65, 0:256], allst[64:65, 256:512], 256, eps_c, am_row, cm_row)
        am_bc = g_bc.tile([128, 256], BF, tag="am_bc", name="am_bc")
        cm_bc = g_bc.tile([128, 256], BF, tag="cm_bc", name="cm_bc")
        nc.gpsimd.partition_broadcast(am_bc[:], am_row[:])
        nc.gpsimd.partition_broadcast(cm_bc[:], cm_row[:])
        hg, hmg = [], []
        for ci in range(NP):
            tmp = g_roll.tile([128, 512], BF, tag="h_tmp", name="h_tmp")
            nc.vector.tensor_mul(tmp[:], xbs[ci][:], a_bc[:])
            hgc = g_roll.tile([128, 512], BF, tag=f"h{ci}", name="hgc")
            nc.vector.tensor_add(hgc[:], tmp[:], c_bc[:])
            hg.append(hgc)
            tmpm = g_roll.tile([128, 256], BF, tag="hm_tmp", name="hm_tmp")
            nc.vector.tensor_mul(tmpm[:], xbms[ci][:], am_bc[:])
            hmgc = g_roll.tile([128, 256], BF, tag=f"hm{ci}", name="hmgc")
            nc.vector.tensor_add(hmgc[:], tmpm[:], cm_bc[:])
            hmg.append(hmgc)
        if "h" in dbg:
            for ci in range(NP):
                nc.sync.dma_start(dbg["h"][ci * 128:(ci + 1) * 128, csl], hg[ci][:])
        if "hm" in dbg:
            for ci in range(NP):
                nc.sync.dma_start(dbg["hm"][ci * 128:(ci + 1) * 128, msl], hmg[ci][:])
        return hg, hmg

    def units_qkv(g, hg, hmg):
        """Per-f / per-t emission units for the K/Q/V GEMMs of group g."""
        csl = slice(g * 512, (g + 1) * 512)
        msl = slice(g * 256, (g + 1) * 256)

        def mk_k(f):
            def fn():
                ps = gps.tile([128, 512], FP, tag="ps", name="ps")
                for c in range(NP):
                    nc.tensor.matmul(ps[:], w_sb[c][:, C + f * 128:C + (f + 1) * 128],
                                     hg[c][:], start=(c == 0), stop=(c == NP - 1))
                nc.scalar.activation(KT[f][:, csl], ps[:], AF.Identity,
                                     bias=bias_sb[f][:, 1:2])
            return fn

        def mk_q(f):
            def fn():
                ps = gps.tile([128, 512], FP, tag="ps", name="ps")
                for c in range(NP):
                    nc.tensor.matmul(ps[:, 0:256], w_sb[c][:, f * 128:(f + 1) * 128],
                                     hmg[c][:], start=(c == 0), stop=(c == NP - 1))
                nc.scalar.activation(QT[f][:, msl], ps[:, 0:256], AF.Identity,
                                     bias=bias_sb[f][:, 0:1])
            return fn

        def mk_v(t):
            def fn():
                v3 = Vsb[4 * g + t][:].rearrange("p (h d) -> p h d", d=65)
                nc.vector.memset(v3[:, :, 64:65], 1.0)
                for fs in range(2):
                    n = 512 if fs == 0 else 256
                    nh = n // 64
                    ps = gps.tile([128, 512], FP, tag="ps", name="ps")
                    for c in range(NP):
                        nc.tensor.matmul(ps[:, 0:n], hg[c][:, t * 128:(t + 1) * 128],
                                         w_sb[c][:, 2 * C + fs * 512:2 * C + fs * 512 + n],
                                         start=(c == 0), stop=(c == NP - 1))
                    nc.vector.scalar_tensor_tensor(
                        v3[:, fs * 8:fs * 8 + nh, 0:64],
                        ps[:, 0:n].rearrange("p (h d) -> p h d", d=64),
                        0.0,
                        bv_bc[:, fs * 512:fs * 512 + n].rearrange("p (h d) -> p h d", d=64),
                        AluOpType.add, AluOpType.add)
            return fn

        return ([mk_k(f) for f in range(NP)] + [mk_q(f) for f in range(NP)]
                + [mk_v(t) for t in range(4)])

    def emit_qkv_group(g, hg, hmg):
        for u in units_qkv(g, hg, hmg):
            u()

    att_cnt = [0]

    def att_head(hh, g, pspool, filler=None, stride=1):
        """Causal attention for head hh over query col group g (512 cols).
        Key blocks processed in pairs sharing one [128,1024] PSUM tile so a
        single exp instruction covers both (3D access pattern)."""
        ht, hp = hh // 2, (hh % 2) * 64
        npairs = 4 + 4 * g
        pa = pa_pool.tile([65, 512], FP, tag="pa", name="pa")
        for u in range(npairs):
            sa, sb = 2 * u, 2 * u + 1
            c0 = max(0, u - 4 * g) * 128
            ps = pspool.tile([128, 1024], FP, tag="ps", name="ps")
            qs = QT[ht][hp:hp + 64, g * 512 + c0:(g + 1) * 512]
            nc.tensor.matmul(ps[:, c0:512],
                             KT[ht][hp:hp + 64, sa * 128:(sa + 1) * 128], qs,
                             start=True, stop=True)
            nc.tensor.matmul(ps[:, 512 + c0:1024],
                             KT[ht][hp:hp + 64, sb * 128:(sb + 1) * 128], qs,
                             start=True, stop=True)
            wei = g_wei.tile([128, 1024], BF, tag="wei", name="wei")
            ps3 = ps[:].rearrange("p (two q) -> p two q", two=2)
            we3 = wei[:].rearrange("p (two q) -> p two q", two=2)
            nc.scalar.activation(we3[:, :, c0:512], ps3[:, :, c0:512], AF.Exp,
                                 bias=shift_c[:])
            jd = u - 4 * g
            if 0 <= jd < 4:
                nc.vector.tensor_mul(
                    we3[:, :, jd * 128:(jd + 1) * 128],
                    we3[:, :, jd * 128:(jd + 1) * 128],
                    mask_ab[:].rearrange("p (two q) -> p two q", two=2))
            nc.tensor.matmul(pa[:, c0:512], Vsb[sa][:, hh * 65:(hh + 1) * 65],
                             wei[:, c0:512], start=(u == 0), stop=False,
                             skip_group_check=True)
            nc.tensor.matmul(pa[:, c0:512], Vsb[sb][:, hh * 65:(hh + 1) * 65],
                             wei[:, 512 + c0:1024], start=False,
                             stop=(u == npairs - 1), skip_group_check=True)
            if filler:
                att_cnt[0] += 1
                if att_cnt[0] % stride == 0:
                    filler.popleft()()
        sumrow = g_asc.tile([1, 512], FP, tag="sumrow", name="sumrow")
        nc.vector.tensor_copy(sumrow[:], pa[64:65, :])
        recip = g_asc.tile([1, 512], FP, tag="recip", name="recip")
        scr = g_asc.tile([1, 512], FP, tag="scr", name="scr")
        nc.vector.reciprocal_approx_accurate(recip[:], sumrow[:], scr[:])
        rb = g_rb.tile([64, 512], FP, tag="rb", name="rb")
        nc.gpsimd.partition_broadcast(rb[:], recip[:])
        nc.vector.tensor_mul(attnT[ht][hp:hp + 64, g * 512:(g + 1) * 512],
                             pa[0:64, :], rb[:])

    # phase 1 pipeline with attention group 0 overlapped onto its tail
    cur = {0: emit_ln_group(0)}
    cur[1] = emit_ln_group(1)
    for c in range(NP):
        nc.sync.dma_start(w_sb[c][:], d_wqkvp[c * 128:(c + 1) * 128, :])
    emit_qkv_group(0, *cur[0])
    cur[2] = emit_ln_group(2)
    emit_qkv_group(1, *cur[1])
    from collections import deque

    def wproj_unit():
        for c in range(NP):
            nc.vector.tensor_copy(wproj[c][:], w_sb[c][:, 3 * C:4 * C])

    fill0 = deque()
    for hh in range(H):
        att_head(hh, 0, ps0_pool, fill0, stride=1)
        if hh == 0:
            cur[3] = emit_ln_group(3)
            fill0.extend(units_qkv(2, *cur[2]))
            fill0.extend(units_qkv(3, *cur[3]))
            fill0.append(wproj_unit)
    while fill0:
        fill0.popleft()()

    for c in range(NP):
        if "KT" in dbg:
            nc.sync.dma_start(dbg["KT"][c * 128:(c + 1) * 128, :], KT[c][:])
        if "QT" in dbg:
            nc.sync.dma_start(dbg["QT"][c * 128:(c + 1) * 128, :], QT[c][:])
    if "V" in dbg:
        for t in range(NT):
            nc.sync.dma_start(dbg["V"][t * 128:(t + 1) * 128, :], Vsb[t][:])
    ln_es.close()
    w_es.close()
    ps0_es.close()

    # ========== fused attention(group 1) + proj + LN2 + MLP ==========
    fus_es = ExitStack()
    g_w1 = fus_es.enter_context(tc.tile_pool(name="w1p", bufs=1))
    g_h2 = fus_es.enter_context(tc.tile_pool(name="h2p", bufs=1))
    g_r = fus_es.enter_context(tc.tile_pool(name="rp", bufs=1))
    g_roll2 = fus_es.enter_context(tc.tile_pool(name="mlproll", bufs=2))
    g_bc2 = fus_es.enter_context(tc.tile_pool(name="mlpbc", bufs=1))
    g_small2 = fus_es.enter_context(tc.tile_pool(name="mlpsmall", bufs=1))
    ps1_pool = fus_es.enter_context(tc.tile_pool(name="sps1", bufs=2, space="PSUM"))
    gps3 = fus_es.enter_context(tc.tile_pool(name="mps", bufs=2, space="PSUM"))

    w1_sb = [g_w1.tile([128, 4 * C], BF, tag=f"w1_{c}", name=f"w1_{c}") for c in range(NP)]
    for c in range(NP):
        nc.sync.dma_start(w1_sb[c][:], d_w1[c * 128:(c + 1) * 128, :])

    def units_proj(g):
        gsl = slice(g * 512, (g + 1) * 512)

        def mk(f):
            def fn():
                ps = gps3.tile([128, 512], FP, tag="ps", name="ps")
                for c in range(NP):
                    nc.tensor.matmul(ps[:], wproj[c][:, f * 128:(f + 1) * 128],
                                     attnT[c][:, gsl],
                                     start=(c == 0), stop=(c == NP - 1))
                xr = g_roll2.tile([128, 512], FP, tag="xr", name="xr")
                nc.sync.dma_start(xr[:], d_xTm[f * 128:(f + 1) * 128, gsl])
                nc.vector.scalar_tensor_tensor(
                    xmid[f][:, gsl], ps[:], bias_sb[f][:, 3:4],
                    xr[:], AluOpType.add, AluOpType.add)
            return fn
        return [mk(f) for f in range(NP)]

    def emit_proj(g):
        for u in units_proj(g):
            u()

    h2 = {}

    def units_ln2(g):
        gsl = slice(g * 512, (g + 1) * 512)
        st = {}
        h2[g] = [None] * NP

        def mk_stat(ci):
            def fn():
                if "stats" not in st:
                    st["stats"] = pa_pool.tile([65, 512], FP, tag="pa", name="st2")
                stats = st["stats"]
                sq = g_roll2.tile([128, 512], BF, tag="sq2", name="sq2")
                nc.vector.tensor_mul(sq[:], xmid[ci][:, gsl], xmid[ci][:, gsl])
                nc.tensor.matmul(stats[0:1, :], ones_bf[:], xmid[ci][:, gsl],
                                 start=(ci == 0), stop=(ci == NP - 1),
                                 skip_group_check=True)
                nc.tensor.matmul(stats[32:33, :], ones_bf[:], sq[:],
                                 start=(ci == 0), stop=(ci == NP - 1),
                                 skip_group_check=True)
            return fn

        def u_smalls():
            stats = st["stats"]
            a_row2 = g_small2.tile([1, 512], BF, tag="a_row2", name="a_row2")
            c_row2 = g_small2.tile([1, 512], BF, tag="c_row2", name="c_row2")
            _ln_smalls(nc, g_small2, stats[0:1, :], stats[32:33, :], 512,
                       eps_c, a_row2, c_row2)
            a_bc2 = g_bc2.tile([128, 512], BF, tag="a2bc", name="a2bc")
            c_bc2 = g_bc2.tile([128, 512], BF, tag="c2bc", name="c2bc")
            nc.gpsimd.partition_broadcast(a_bc2[:], a_row2[:])
            nc.gpsimd.partition_broadcast(c_bc2[:], c_row2[:])
            st["a_bc2"], st["c_bc2"] = a_bc2, c_bc2

        def mk_h2(ci):
            def fn():
                tmp = g_roll2.tile([128, 512], BF, tag="h2tmp", name="h2tmp")
                nc.vector.tensor_mul(tmp[:], xmid[ci][:, gsl], st["a_bc2"][:])
                hh2 = g_h2.tile([128, 512], BF, tag=f"h2_{ci}", name=f"h2_{ci}")
                nc.vector.tensor_add(hh2[:], tmp[:], st["c_bc2"][:])
                h2[g][ci] = hh2
            return fn

        return ([mk_stat(ci) for ci in range(NP)] + [u_smalls]
                + [mk_h2(ci) for ci in range(NP)])

    def emit_ln2(g):
        for u in units_ln2(g):
            u()

    r_tiles = {}

    def units_fc1(g):
        def mk(m):
            def fn():
                ps = gps3.tile([128, 512], FP, tag="ps", name="ps")
                for c in range(NP):
                    nc.tensor.matmul(ps[:], w1_sb[c][:, m * 128:(m + 1) * 128],
                                     h2[g][c][:],
                                     start=(c == 0), stop=(c == NP - 1))
                r = g_r.tile([128, 512], BF, tag=f"r{m}", name=f"r{m}")
                nc.vector.tensor_scalar(r[:], ps[:],
                                        bias_sb[m % 6][:, 5 + m // 6:6 + m // 6],
                                        0.0, AluOpType.add, AluOpType.max)
                r_tiles[g, m] = r
            return fn
        return [mk(m) for m in range(24)]

    def emit_fc1(g):
        for u in units_fc1(g):
            u()

    w2k = [None] * 6
    w2q = [None] * 6

    def emit_w2_chunk(i):
        """Load w2 columns into the KT[i]/QT[i] buffers (pool-tag reuse) as
        soon as heads 2i/2i+1 have finished reading them."""
        w2k[i] = g_kqv.tile([128, 2048], BF, tag=f"KT{i}", name=f"w2k{i}")
        g0, g1 = i * 2048, (i + 1) * 2048
        st = g0
        while st < g1:
            c = st // 3072
            en = min(g1, (c + 1) * 3072)
            nc.sync.dma_start(w2k[i][:, st - g0:en - g0],
                              d_w2[c * 128:(c + 1) * 128,
                                   st - c * 3072:en - c * 3072])
            st = en
        w2q[i] = g_kqv.tile([128, 1024], BF, tag=f"QT{i}", name=f"w2q{i}")
        q0 = 12288 + 1024 * i
        c = q0 // 3072
        nc.sync.dma_start(w2q[i][:], d_w2[c * 128:(c + 1) * 128,
                                          q0 - c * 3072:q0 - c * 3072 + 1024])

    def w2_slice(m, f):
        g0 = (m // 4) * 3072 + (m % 4) * 768 + f * 128
        if g0 < 12288:
            return w2k[g0 // 2048][:, g0 % 2048:g0 % 2048 + 128]
        rr = g0 - 12288
        return w2q[rr // 1024][:, rr % 1024:rr % 1024 + 128]

    def emit_fc2(g):
        gsl = slice(g * 512, (g + 1) * 512)
        for f in range(NP):
            ps = gps3.tile([128, 512], FP, tag="ps", name="ps")
            for m in range(24):
                nc.tensor.matmul(ps[:], w2_slice(m, f), r_tiles[g, m][:],
                                 start=(m == 0), stop=(m == 23))
            ot = g_roll2.tile([128, 512], FP, tag="ot", name="ot")
            nc.vector.scalar_tensor_tensor(ot[:], ps[:], bias_sb[f][:, 4:5],
                                           xmid[f][:, gsl],
                                           AluOpType.add, AluOpType.add)
            nc.sync.dma_start(d_out[f * 128:(f + 1) * 128, gsl], ot[:])

    # attention group 1 interleaved (one small unit per two key-block pairs)
    # with proj/LN2/fc1 of group 0 so the PE keeps dense work and full clock
    # while the exp chain runs; w2 loads ride the freed KT/QT buffers
    fill1 = deque(units_proj(0) + units_ln2(0) + units_fc1(0))
    for hh in range(H):
        att_head(hh, 1, ps1_pool, fill1, stride=2)
        if hh % 2 == 1:
            emit_w2_chunk(hh // 2)
    while fill1:
        fill1.popleft()()
    if "attnT" in dbg:
        for c in range(NP):
            nc.sync.dma_start(dbg["attnT"][c * 128:(c + 1) * 128, :], attnT[c][:])

    emit_proj(1)
    emit_ln2(1)
    emit_fc2(0)
    emit_fc1(1)
    emit_fc2(1)
    if "xmid" in dbg:
        for c in range(NP):
            nc.sync.dma_start(dbg["xmid"][c * 128:(c + 1) * 128, :], xmid[c][:])
    fus_es.close()
    fusA_es.close()
    attnT_es.close()
    kqv_stack.close()
    es.close()


# ---------------------------------------------------------------------------
# host side
# ---------------------------------------------------------------------------

def _mycols(half):
    blocks = np.arange(8) * 2 + half
    return (blocks[:, None] * 128 + np.arange(128)[None, :]).reshape(-1)


def _prep_inputs(x, wq, bq, wk, bk, wv, bv, w_proj, b_proj, w1, b1, w2, b2,
                 g1, beta1, g2, beta2):
    x = np.asarray(x, np.float32)
    wq_f = np.ascontiguousarray(np.transpose(np.asarray(wq, np.float32), (1, 0, 2)).reshape(C, C))
    wk_f = np.ascontiguousarray(np.transpose(np.asarray(wk, np.float32), (1, 0, 2)).reshape(C, C))
    wv_f = np.ascontiguousarray(np.transpose(np.asarray(wv, np.float32), (1, 0, 2)).reshape(C, C))
    g1 = np.asarray(g1, np.float32); beta1 = np.asarray(beta1, np.float32)
    g2 = np.asarray(g2, np.float32); beta2 = np.asarray(beta2, np.float32)
    w1 = np.asarray(w1, np.float32); w2 = np.asarray(w2, np.float32)
    w_proj = np.asarray(w_proj, np.float32)

    wq_g = g1[:, None] * wq_f
    wk_g = g1[:, None] * wk_f
    wv_g = g1[:, None] * wv_f
    bq_f = beta1 @ wq_f + np.asarray(bq, np.float32).reshape(-1)
    bk_f = beta1 @ wk_f + np.asarray(bk, np.float32).reshape(-1)
    bv_f = beta1 @ wv_f + np.asarray(bv, np.float32).reshape(-1)
    w1_g = g2[:, None] * w1
    b1_f = beta2 @ w1 + np.asarray(b1, np.float32)

    wqkvp = np.concatenate([wq_g, wk_g, wv_g, w_proj], axis=1).astype(bf16)
    w1p = w1_g.astype(bf16)
    w2p = np.ascontiguousarray(
        w2.reshape(6, 4, 128, C).transpose(0, 2, 1, 3).reshape(C, 4 * C)).astype(bf16)

    biasp = np.zeros((C, 9), np.float32)
    biasp[:, 0] = bq_f
    biasp[:, 1] = bk_f
    biasp[:, 2] = bv_f
    biasp[:, 3] = np.asarray(b_proj, np.float32)
    biasp[:, 4] = np.asarray(b2, np.float32)
    biasp[:, 5:9] = b1_f.reshape(4, C).T
    bvrow = bv_f.reshape(1, C).astype(np.float32)

    tri = np.tril(np.ones((128, 128), np.float32)).T  # [s, q]: 1 iff s <= q
    in_maps = []
    for core in range(8):
        b, half = core // 2, core % 2
        xT = np.ascontiguousarray(x[b].T)
        xTm = np.ascontiguousarray(xT[:, _mycols(half)])
        masks = np.zeros((256, 128), np.float32)
        if half == 0:
            masks[0:128] = tri
            masks[128:256] = 0.0
        else:
            masks[0:128] = 1.0
            masks[128:256] = tri
        in_maps.append({
            "xT": xT, "xTm": xTm,
            "wqkvp": wqkvp, "w1p": w1p, "w2p": w2p,
            "biasp": biasp, "bvrow": bvrow,
            "masks": masks.astype(bf16),
        })
    return in_maps


def _assemble(results, dtype):
    out = np.empty((B, T, C), dtype)
    for core in range(8):
        b, half = core // 2, core % 2
        out[b, _mycols(half), :] = results[core]["outT"].T
    return out


def kernel(**inputs):
    in_maps = _prep_inputs(**inputs)
    if "nc" not in _cache:
        _cache["nc"] = _build()
    res = bass_utils.run_bass_kernel_spmd(_cache["nc"], in_maps,
                                          core_ids=list(range(8)))
    return _assemble(res.results, np.asarray(inputs["x"]).dtype)
